# revision 17
# baseline (speedup 1.0000x reference)
"""GNN message passing (nn_NodeToNode) on 8 trn2 NeuronCores via Bass/Tile.

Algorithm (per core, SPMD):
  - Nodes are range-sharded: core c owns nodes [c*6272, (c+1)*6272) (50176 total,
    padded; host slices output back to 50000).
  - Host sorts the doubled edge list by receiver and buckets edges into the
    owner core's 49 node-blocks of 128. Within each block bucket, edges are
    split into a lo stream (sender < 32768) and a hi stream (sender >= 32768)
    because dma_gather indices are int16. Each stream is padded to whole
    128-edge chunks (pad: sender-slot 0, rloc=-1).
  - Phase 1 on device, per 4-block group: TWO batched SWDGE dma_gather calls
    (prepare_only + trigger_dma; lo chunks from x[0:32768], hi chunks from
    x[32768:], indices int16 wrapped [n%16, n//16] and replicated over the 8
    Q7 replica partition groups) fetch all sender rows of bf16 x; ONE batched
    DVE is_equal per stream builds the one-hot S[e, col, n] = (iota[n] ==
    rloc[e, col]); per block, bf16 matmuls accumulate aggT[f, n] += M^T . S
    in PSUM over the block's lo+hi chunk columns. rloc=-1 padding zeroes the
    S rows, masking pad/garbage lanes. PE waits on the gather DMA-completion
    semaphores explicitly (tile does not wait for prepare_only DMA data).
    Batching ~70 chunks per gather call amortizes the ~1us fixed SWDGE cost
    per DMA instruction that dominated the unbatched version.
  - Phase 2 on device (transposed layout, per 512-node group, emitted right
    after its 4 phase-1 blocks for overlap): 3-layer MLP with per-partition
    biases on ACT (exact-erf GELU) with bf16 matmul inputs, LayerNorm over
    the feature (=partition) axis in fp32 via ones-matmul stats +
    replicate-matmul broadcast, then PE transpose back to [node, feat].
"""
import os
import sys
import types
import contextlib
import ctypes

import numpy as np
import ml_dtypes

import concourse.bacc as bacc
import concourse.mybir as mybir
import concourse.tile as tile
from concourse.instruction_name_ordered_set import InstructionNameOrderedSet
from concourse.bass_utils import run_bass_kernel_spmd
from concourse.masks import make_identity

P = 128
N_NODES = 50000
SPLIT = 32768               # lo/hi sender split (int16 index limit)
D_IN = 128
D_HID = 256
D_OUT = 128
N_CORES = 8
NB = 49                     # real node blocks per core
NBD = 50                    # device blocks (block 0 is a sacrificial pad block)
NPC = NB * P                # real nodes per core (6272), 8*6272 = 50176 >= 50000
NPCD = NBD * P              # device rows per core (6400)
N_PAD = N_CORES * NPC
GB = 4                      # blocks per gather/MLP group
NSEM = 8                    # rotating gather-completion semaphores

F32 = mybir.dt.float32
BF16 = mybir.dt.bfloat16
I16 = mybir.dt.int16

_LAST_EXEC_NS = None        # set when BASS_GNN_TRACE=1
_LAST_RESULTS = None


# ---------------------------------------------------------------------------
# NTFF profiling hook (only used when BASS_GNN_TRACE=1); injects the missing
# antenv.axon_hooks module using ctypes against libaxon_pjrt.so.
# ---------------------------------------------------------------------------
def _install_ntff_hook():
    so = "/opt/axon/libaxon_pjrt.so"
    if "antenv.axon_hooks" in sys.modules or not os.path.exists(so):
        return
    lib = ctypes.CDLL(so)
    if not hasattr(lib, "axon_start_nrt_profile"):
        return
    lib.axon_start_nrt_profile.argtypes = [ctypes.POINTER(ctypes.c_int64), ctypes.c_size_t]
    lib.axon_start_nrt_profile.restype = ctypes.c_int64
    lib.axon_stop_nrt_profile.argtypes = [ctypes.c_char_p]
    lib.axon_stop_nrt_profile.restype = ctypes.c_int64

    @contextlib.contextmanager
    def _hook(output_dir, device_ids):
        import jax

        jax.devices()
        if device_ids:
            ids = (ctypes.c_int64 * len(device_ids))(*device_ids)
            rc = lib.axon_start_nrt_profile(ids, len(device_ids))
        else:
            rc = lib.axon_start_nrt_profile(None, 0)
        if rc != 0:
            raise RuntimeError(f"axon_start_nrt_profile rc={rc}")
        try:
            yield
        finally:
            n = lib.axon_stop_nrt_profile(str(output_dir).encode())
            print(f"profile: {n} ntff file(s) -> {output_dir}", file=sys.stderr)

    mod = types.ModuleType("antenv.axon_hooks")
    mod.get_axon_ntff_profile_hook = lambda: _hook
    mod.set_axon_ntff_profile_hook = lambda h: None
    sys.modules["antenv.axon_hooks"] = mod


def _groups():
    out = []
    b = 0
    while b < NBD:
        nb = min(GB, NBD - b)
        out.append((b, nb))
        b += nb
    return out


# ---------------------------------------------------------------------------
# Host-side edge preprocessing
# ---------------------------------------------------------------------------
def _preprocess(edge_index):
    """Bucket doubled edges by destination block, split lo/hi by sender, and
    build per-core int16 gather-index (wrapped) + local-receiver tiles.

    Returns (idx16_tiles[c], rloc_tiles[c], layout) where layout carries the
    per-block chunk-column ranges and per-group gather-call extents.
    """
    send = np.concatenate([edge_index[0], edge_index[1]]).astype(np.int64)
    recv = np.concatenate([edge_index[1], edge_index[0]]).astype(np.int64)

    blk = recv // P                          # global block id, 0..391
    hi = (send >= SPLIT).astype(np.int64)
    order = np.lexsort((hi, blk))            # by block, lo before hi
    send_s = send[order]
    recv_s = recv[order]
    blk_s = blk[order]
    hi_s = hi[order]

    n_blk_glob = N_PAD // P                  # 392
    counts = np.bincount(blk_s, minlength=n_blk_glob)
    nlo_g = np.bincount(blk_s[hi_s == 0], minlength=n_blk_glob)
    nhi_g = counts - nlo_g
    nlo_cb = nlo_g.reshape(N_CORES, NB)
    nhi_cb = nhi_g.reshape(N_CORES, NB)
    Klo_r = np.maximum(np.ceil(nlo_cb.max(axis=0) / P).astype(np.int64), 1)
    Khi_r = np.ceil(nhi_cb.max(axis=0) / P).astype(np.int64)
    # device block 0 is sacrificial: one all-pad lo chunk, no hi chunks
    Klo = np.concatenate([[1], Klo_r])
    Khi = np.concatenate([[0], Khi_r])

    # column layout: per group, lo chunks of its blocks then hi chunks
    lo_start = np.zeros(NBD, np.int64)
    hi_start = np.zeros(NBD, np.int64)
    g_meta = []
    col = 0
    for b0, nb in _groups():
        g_col0 = col
        for b in range(b0, b0 + nb):
            lo_start[b] = col
            col += Klo[b]
        lo_cols = col - g_col0
        for b in range(b0, b0 + nb):
            hi_start[b] = col
            col += Khi[b]
        hi_cols = col - g_col0 - lo_cols
        g_meta.append((g_col0, lo_cols, hi_cols))
        assert lo_cols * P <= 15000 and hi_cols * P <= 15000, (
            "gather call exceeds Q7 idx scratch"
        )
    TOT = int(col)

    starts = np.concatenate([[0], np.cumsum(counts)])
    # rank of each edge within its (block, stream) segment
    j_all = np.arange(send_s.shape[0]) - starts[blk_s]
    j_seg = np.where(hi_s == 0, j_all, j_all - nlo_g[blk_s])

    b_local = blk_s % NB + 1     # device block index (0 is sacrificial)
    seg_start = np.where(hi_s == 0, lo_start[b_local], hi_start[b_local])
    col_e = seg_start + j_seg // P
    lane_e = j_seg % P
    val_e = np.where(hi_s == 0, send_s, send_s - SPLIT).astype(np.int16)
    rloc_e = (recv_s - (blk_s * P)).astype(np.float32)

    idx16_tiles, rloc_tiles = [], []
    n_wrap = np.arange(TOT * P)
    for c in range(N_CORES):
        lo, hic = starts[c * NB], starts[(c + 1) * NB]
        sl = slice(lo, hic)
        flat = np.zeros(TOT * P, dtype=np.int16)
        flat[col_e[sl] * P + lane_e[sl]] = val_e[sl]
        idx16 = np.zeros((P, TOT * 8), dtype=np.int16)
        for r in range(8):
            idx16[16 * r + (n_wrap % 16), n_wrap // 16] = flat
        rloc_t = np.full((P, TOT), -1.0, dtype=np.float32)
        rloc_t[lane_e[sl], col_e[sl]] = rloc_e[sl]
        idx16_tiles.append(idx16)
        rloc_tiles.append(rloc_t.astype(ml_dtypes.bfloat16))

    layout = (Klo, Khi, lo_start, hi_start, g_meta, TOT)
    return idx16_tiles, rloc_tiles, layout


# ---------------------------------------------------------------------------
# Kernel build
# ---------------------------------------------------------------------------
def _build(layout):
    Klo, Khi, lo_start, hi_start, g_meta, TOT = layout
    nc = bacc.Bacc("TRN2", target_bir_lowering=False, debug=False, num_devices=N_CORES)

    x = nc.declare_dram_parameter("x", [N_NODES, D_IN], BF16, isOutput=False)
    idx = nc.declare_dram_parameter("idx", [P, TOT * 8], I16, isOutput=False)
    # rl = iota (128 cols) | rloc (TOT cols)
    rl = nc.declare_dram_parameter("rl", [P, P + TOT], BF16, isOutput=False)
    metaf = nc.declare_dram_parameter("metaf", [P, 7], F32, isOutput=False)
    w1 = nc.declare_dram_parameter("w1", [D_IN, D_HID], BF16, isOutput=False)
    w2 = nc.declare_dram_parameter("w2", [D_HID, D_HID], BF16, isOutput=False)
    w3 = nc.declare_dram_parameter("w3", [D_HID, D_OUT], BF16, isOutput=False)
    out = nc.declare_dram_parameter("out", [NPCD, D_OUT], F32, isOutput=True)
    aggdbg = nc.declare_dram_parameter("aggdbg", [P, NPCD], F32, isOutput=True)

    AF = mybir.ActivationFunctionType
    OP = mybir.AluOpType

    with tile.TileContext(nc) as tc:
        sems = [nc.alloc_semaphore(f"gsem{i}") for i in range(NSEM)]
        uses = [0] * NSEM
        sem_rot = [0]  # next sem index

        with (
            tc.tile_pool(name="const", bufs=1) as cpool,
            tc.tile_pool(name="gather", bufs=2) as gpool,
            tc.tile_pool(name="spool", bufs=2) as spool,
            tc.tile_pool(name="agg", bufs=1) as apool,
            tc.tile_pool(name="dbg", bufs=1) as dbgpool,
            tc.tile_pool(name="hid", bufs=8) as hpool,
            tc.tile_pool(name="rows", bufs=6) as rpool,
            tc.tile_pool(name="outp", bufs=4) as opool,
            tc.tile_pool(name="ps1", bufs=2, space="PSUM") as ps1pool,
            tc.tile_pool(name="ps2", bufs=4, space="PSUM") as ps2pool,
            tc.tile_pool(name="psr", bufs=2, space="PSUM") as psrpool,
        ):
            # ---- constants -------------------------------------------------
            idx_sb = cpool.tile([P, TOT * 8], I16)
            nc.sync.dma_start(out=idx_sb[:], in_=idx[:])
            rl_sb = cpool.tile([P, P + TOT], BF16)
            nc.sync.dma_start(out=rl_sb[:], in_=rl[:])
            iota_sb = rl_sb[:, 0:P]
            rloc_sb = rl_sb[:, P : P + TOT]

            metaf_sb = cpool.tile([P, 7], F32)
            nc.sync.dma_start(out=metaf_sb[:], in_=metaf[:])
            b1_ap = metaf_sb[:, 0:2]
            b2_ap = metaf_sb[:, 2:4]
            b3_ap = metaf_sb[:, 4:5]
            lng_ap = metaf_sb[:, 5:6]
            lnb_ap = metaf_sb[:, 6:7]

            w1_sb = cpool.tile([P, D_HID], BF16)
            nc.sync.dma_start(out=w1_sb[:], in_=w1[:])
            w2_sb = cpool.tile([P, 2 * D_HID], BF16)
            nc.sync.dma_start(
                out=w2_sb[:].rearrange("p (h j) -> p h j", h=2),
                in_=w2[:].rearrange("(h p) j -> p h j", p=P),
            )
            w3_sb = cpool.tile([P, 2 * D_OUT], BF16)
            nc.sync.dma_start(
                out=w3_sb[:].rearrange("p (h j) -> p h j", h=2),
                in_=w3[:].rearrange("(h p) j -> p h j", p=P),
            )

            ident_sb = cpool.tile([P, P], F32)
            make_identity(nc, ident_sb[:])
            ones_col = cpool.tile([P, 1], F32)
            nc.vector.memset(ones_col[:], 1.0)
            ones_row = cpool.tile([1, P], F32)
            nc.vector.memset(ones_row[:], 1.0)

            aggB = apool.tile([P, NPCD], BF16)   # [feat, node] for this core

            # Pool-engine touch of idx_sb: gives the gather preps (whose
            # metadata read tile does not gate on the load DMA completion)
            # a properly-waited predecessor in Pool program order
            idx_tok = cpool.tile([P, 8], I16)
            nc.gpsimd.tensor_copy(out=idx_tok[:], in_=idx_sb[:, 0:8])

            # sacrificial warmup gather: the first ext-ISA dma_gather after
            # the IRAM library load misbehaves; absorb it with a dummy call
            warm = cpool.tile([P, P], BF16)
            nc.gpsimd.dma_start(out=warm[:], in_=x[0:P, :])

            # the SWDGE descriptor ring holds ~64 descs per DMA engine and a
            # single prep must fit it whole: cap each call at 7 chunk columns
            # (896 idxs -> 57 descs/DMA incl. sem)
            CALL_COLS = 7

            def gather_call(mt, dst_c0, cols, src_lo, g_col0):
                """Batched dma_gather of `cols` chunk columns, split into
                ring-sized prep+trigger sub-calls. Returns (sem, target)s."""
                waits = []
                done = 0
                while done < cols:
                    cc = min(CALL_COLS, cols - done)
                    s = sem_rot[0]
                    sem_rot[0] = (s + 1) % NSEM
                    c0 = dst_c0 + done
                    nc.gpsimd.dma_gather(
                        out_ap=mt[:, c0 * P : (c0 + cc) * P].rearrange(
                            "p (k j) -> p k j", k=cc
                        ),
                        in_ap=x[0:SPLIT, :] if src_lo else x[SPLIT:N_NODES, :],
                        idxs_ap=idx_sb[:, (g_col0 + done) * 8 : (g_col0 + done + cc) * 8],
                        num_idxs=cc * P,
                        num_idxs_reg=cc * P,
                        elem_size=D_IN,
                        prepare_only=True,
                        sem=sems[s],
                    )
                    nc.gpsimd.trigger_dma(count=None)
                    uses[s] += 1
                    waits.append((s, 16 * uses[s]))
                    done += cc
                return waits

            # ---- phase 2: transposed MLP + LayerNorm on a node group -------
            def phase2_group(g0, ng):
                rhs_agg = aggB[:, g0 : g0 + ng]
                h1 = []
                for jh in range(2):
                    p1 = ps2pool.tile([P, ng], F32, tag="p2")
                    nc.tensor.matmul(
                        out=p1[:], lhsT=w1_sb[:, jh * P : (jh + 1) * P],
                        rhs=rhs_agg, start=True, stop=True,
                    )
                    t = hpool.tile([P, ng], BF16, tag="h")
                    nc.scalar.activation(t[:], p1[:], AF.Gelu, bias=b1_ap[:, jh : jh + 1])
                    h1.append(t)
                h2 = []
                for kh in range(2):
                    p2 = ps2pool.tile([P, ng], F32, tag="p2")
                    for jh in range(2):
                        nc.tensor.matmul(
                            out=p2[:],
                            lhsT=w2_sb[:, jh * D_HID + kh * P : jh * D_HID + (kh + 1) * P],
                            rhs=h1[jh][:], start=(jh == 0), stop=(jh == 1),
                        )
                    t = hpool.tile([P, ng], BF16, tag="h")
                    nc.scalar.activation(t[:], p2[:], AF.Gelu, bias=b2_ap[:, kh : kh + 1])
                    h2.append(t)
                p3 = ps2pool.tile([P, ng], F32, tag="p2")
                for kh in range(2):
                    nc.tensor.matmul(
                        out=p3[:], lhsT=w3_sb[:, kh * D_OUT : (kh + 1) * D_OUT],
                        rhs=h2[kh][:], start=(kh == 0), stop=(kh == 1),
                    )
                h3 = hpool.tile([P, ng], F32, tag="hf")
                nc.scalar.activation(h3[:], p3[:], AF.Identity, bias=b3_ap)
                sq = hpool.tile([P, ng], F32, tag="hf")
                nc.scalar.activation(sq[:], h3[:], AF.Square)

                mu_ps = psrpool.tile([1, ng], F32, tag="pr")
                nc.tensor.matmul(out=mu_ps[:], lhsT=ones_col[:], rhs=h3[:], start=True, stop=True)
                s2_ps = psrpool.tile([1, ng], F32, tag="pr")
                nc.tensor.matmul(out=s2_ps[:], lhsT=ones_col[:], rhs=sq[:], start=True, stop=True)

                m_row = rpool.tile([1, ng], F32, tag="r")
                nc.vector.tensor_scalar_mul(m_row[:], mu_ps[:], 1.0 / P)
                q_row = rpool.tile([1, ng], F32, tag="r")
                nc.vector.tensor_tensor(out=q_row[:], in0=m_row[:], in1=m_row[:], op=OP.mult)
                v_row = rpool.tile([1, ng], F32, tag="r")
                nc.vector.tensor_scalar_mul(v_row[:], s2_ps[:], 1.0 / P)
                nc.vector.tensor_tensor(out=v_row[:], in0=v_row[:], in1=q_row[:], op=OP.subtract)
                nc.vector.tensor_scalar_add(v_row[:], v_row[:], 1e-5)
                sdev = rpool.tile([1, ng], F32, tag="r")
                nc.scalar.activation(sdev[:], v_row[:], AF.Sqrt)
                inv_row = rpool.tile([1, ng], F32, tag="r")
                with nc.allow_low_precision("matching jax rsqrt f32"):
                    nc.vector.reciprocal(inv_row[:], sdev[:])
                minv_row = rpool.tile([1, ng], F32, tag="r")
                nc.vector.tensor_tensor(out=minv_row[:], in0=m_row[:], in1=inv_row[:], op=OP.mult)

                inv_ps = ps2pool.tile([P, ng], F32, tag="p2")
                nc.tensor.matmul(out=inv_ps[:], lhsT=ones_row[:], rhs=inv_row[:], start=True, stop=True)
                minv_ps = ps2pool.tile([P, ng], F32, tag="p2")
                nc.tensor.matmul(out=minv_ps[:], lhsT=ones_row[:], rhs=minv_row[:], start=True, stop=True)

                t1 = hpool.tile([P, ng], F32, tag="hf")
                nc.vector.tensor_tensor(out=t1[:], in0=h3[:], in1=inv_ps[:], op=OP.mult)
                t2 = hpool.tile([P, ng], F32, tag="hf")
                nc.vector.tensor_tensor(out=t2[:], in0=t1[:], in1=minv_ps[:], op=OP.subtract)
                oT = hpool.tile([P, ng], F32, tag="hf")
                nc.vector.tensor_scalar(
                    out=oT[:], in0=t2[:], scalar1=lng_ap, scalar2=lnb_ap,
                    op0=OP.mult, op1=OP.add,
                )

                for t in range(ng // P):
                    trp = ps2pool.tile([P, P], F32, tag="p2")
                    nc.tensor.transpose(out=trp[:], in_=oT[:, t * P : (t + 1) * P], identity=ident_sb[:])
                    ot = opool.tile([P, P], F32, tag="o")
                    nc.scalar.copy(out=ot[:], in_=trp[:])
                    r0 = g0 + t * P
                    nc.sync.dma_start(out=out[r0 : r0 + P, :], in_=ot[:])

            # ---- main loop: gather group -> blocks -> MLP group ------------
            for gi, (b0, nb) in enumerate(_groups()):
                g_col0, lo_cols, hi_cols = g_meta[gi]
                totg = lo_cols + hi_cols
                mt = gpool.tile([P, totg * P], BF16, tag="m")
                w_lo = gather_call(mt, 0, lo_cols, True, g_col0)
                w_hi = gather_call(mt, lo_cols, hi_cols, False, g_col0 + lo_cols)

                # gather data is consumed by PE via the S-build's engine:
                # explicit DVE waits on the DMA completion sems before the
                # one-hot build (tile only orders against the prep); the PE
                # matmuls read S, so they transitively run after the waits.
                # dedupe by sem, keeping the highest target
                needed = {}
                for s_i, tgt in w_lo + w_hi:
                    needed[s_i] = max(needed.get(s_i, 0), tgt)
                w_insts = [
                    nc.vector.wait_ge(sems[s_i], tgt) for s_i, tgt in needed.items()
                ]

                # batched one-hot build over the whole group's columns; pinned
                # after the waits so the PE matmuls (which wait on this build)
                # transitively wait for the gather DMA completions
                s = spool.tile([P, totg * P], BF16, tag="s")
                s_call = nc.vector.tensor_tensor(
                    out=s[:].rearrange("p (k j) -> p k j", k=totg),
                    in0=iota_sb.unsqueeze(1).broadcast_to([P, totg, P]),
                    in1=rloc_sb[:, g_col0 : g_col0 + totg].unsqueeze(2).broadcast_to(
                        [P, totg, P]
                    ),
                    op=OP.is_equal,
                )
                dep_set = InstructionNameOrderedSet()
                for w in w_insts:
                    dep_set.add(w.ins.name)
                if len(dep_set):
                    s_call.ins.add_nosync_dependencies_from(dep_set)

                for b in range(b0, b0 + nb):
                    cols = []
                    cols.extend(range(int(lo_start[b]) - g_col0, int(lo_start[b]) - g_col0 + int(Klo[b])))
                    cols.extend(range(int(hi_start[b]) - g_col0, int(hi_start[b]) - g_col0 + int(Khi[b])))
                    ps = ps1pool.tile([P, P], F32, tag="p1")
                    for ki, k in enumerate(cols):
                        nc.tensor.matmul(
                            out=ps[:],
                            lhsT=mt[:, k * D_IN : (k + 1) * D_IN],
                            rhs=s[:, k * P : (k + 1) * P],
                            start=(ki == 0),
                            stop=(ki == len(cols) - 1),
                        )
                    nc.scalar.copy(out=aggB[:, b * P : (b + 1) * P], in_=ps[:])

                phase2_group(b0 * P, nb * P)
            agf = dbgpool.tile([P, NPCD], F32)
            nc.vector.tensor_copy(out=agf[:], in_=aggB[:])
            nc.sync.dma_start(out=aggdbg[:], in_=agf[:])
    nc.compile()
    return nc


# ---------------------------------------------------------------------------
# Public entry point
# ---------------------------------------------------------------------------
def kernel(x, edge_index, W1, b1, W2, b2, W3, b3, ln_g, ln_b):
    global _LAST_EXEC_NS
    x_bf = np.ascontiguousarray(
        np.asarray(x, dtype=np.float32).astype(ml_dtypes.bfloat16)
    )
    edge_index = np.asarray(edge_index)

    idx16_tiles, rloc_tiles, layout = _preprocess(edge_index)
    TOT = layout[-1]

    iota = np.tile(np.arange(P, dtype=np.float32), (P, 1)).astype(ml_dtypes.bfloat16)
    b1_2 = np.asarray(b1, np.float32).reshape(2, P).T          # [128, 2]
    b2_2 = np.asarray(b2, np.float32).reshape(2, P).T
    b3_1 = np.asarray(b3, np.float32).reshape(1, P).T          # [128, 1]
    g_1 = np.asarray(ln_g, np.float32).reshape(1, P).T
    lb_1 = np.asarray(ln_b, np.float32).reshape(1, P).T
    metaf = np.ascontiguousarray(
        np.concatenate([b1_2, b2_2, b3_1, g_1, lb_1], axis=1).astype(np.float32)
    )

    w1_bf = np.ascontiguousarray(np.asarray(W1, np.float32).astype(ml_dtypes.bfloat16))
    w2_bf = np.ascontiguousarray(np.asarray(W2, np.float32).astype(ml_dtypes.bfloat16))
    w3_bf = np.ascontiguousarray(np.asarray(W3, np.float32).astype(ml_dtypes.bfloat16))

    in_maps = []
    for c in range(N_CORES):
        rl_c = np.ascontiguousarray(np.concatenate([iota, rloc_tiles[c]], axis=1))
        in_maps.append(
            {
                "x": x_bf,
                "idx": idx16_tiles[c],
                "rl": rl_c,
                "metaf": metaf,
                "w1": w1_bf,
                "w2": w2_bf,
                "w3": w3_bf,
            }
        )

    nc = _build(layout)

    trace = os.environ.get("BASS_GNN_TRACE", "0") == "1"
    if trace:
        _install_ntff_hook()
    r = run_bass_kernel_spmd(nc, in_maps, list(range(N_CORES)), trace=trace)
    _LAST_EXEC_NS = r.exec_time_ns

    global _LAST_RESULTS
    _LAST_RESULTS = r.results
    full = np.concatenate(
        [r.results[c]["out"][P:NPCD] for c in range(N_CORES)], axis=0
    )
    return np.ascontiguousarray(full[:N_NODES].astype(np.float32))


# revision 20
# speedup vs baseline: 1.0056x; 1.0056x over previous
"""GNN message passing (nn_NodeToNode) on 8 trn2 NeuronCores via Bass/Tile.

Algorithm (per core, SPMD):
  - Nodes are range-sharded: core c owns nodes [c*6272, (c+1)*6272) (50176 total,
    padded; host slices output back to 50000).
  - Host sorts the doubled edge list by receiver and buckets edges into the
    owner core's 49 node-blocks of 128. Within each block bucket, edges are
    split into a lo stream (sender < 32768) and a hi stream (sender >= 32768)
    because dma_gather indices are int16. Each stream is padded to whole
    128-edge chunks (pad: sender-slot 0, rloc=-1).
  - Phase 1 on device, per 4-block group: TWO batched SWDGE dma_gather calls
    (prepare_only + trigger_dma; lo chunks from x[0:32768], hi chunks from
    x[32768:], indices int16 wrapped [n%16, n//16] and replicated over the 8
    Q7 replica partition groups) fetch all sender rows of bf16 x; ONE batched
    DVE is_equal per stream builds the one-hot S[e, col, n] = (iota[n] ==
    rloc[e, col]); per block, bf16 matmuls accumulate aggT[f, n] += M^T . S
    in PSUM over the block's lo+hi chunk columns. rloc=-1 padding zeroes the
    S rows, masking pad/garbage lanes. PE waits on the gather DMA-completion
    semaphores explicitly (tile does not wait for prepare_only DMA data).
    Batching ~70 chunks per gather call amortizes the ~1us fixed SWDGE cost
    per DMA instruction that dominated the unbatched version.
  - Phase 2 on device (transposed layout, per 512-node group, emitted right
    after its 4 phase-1 blocks for overlap): 3-layer MLP with per-partition
    biases on ACT (exact-erf GELU) with bf16 matmul inputs, LayerNorm over
    the feature (=partition) axis in fp32 via ones-matmul stats +
    replicate-matmul broadcast, then PE transpose back to [node, feat].
"""
import os
import sys
import types
import contextlib
import ctypes

import numpy as np
import ml_dtypes

import concourse.bacc as bacc
import concourse.mybir as mybir
import concourse.tile as tile
from concourse.instruction_name_ordered_set import InstructionNameOrderedSet
from concourse.bass_utils import run_bass_kernel_spmd
from concourse.masks import make_identity

P = 128
N_NODES = 50000
SPLIT = 32768               # lo/hi sender split (int16 index limit)
D_IN = 128
D_HID = 256
D_OUT = 128
N_CORES = 8
NB = 49                     # real node blocks per core
NBD = 50                    # device blocks (block 0 is a sacrificial pad block)
NPC = NB * P                # real nodes per core (6272), 8*6272 = 50176 >= 50000
NPCD = NBD * P              # device rows per core (6400)
N_PAD = N_CORES * NPC
GB = 4                      # blocks per gather/MLP group
NSEM = 8                    # rotating gather-completion semaphores

F32 = mybir.dt.float32
BF16 = mybir.dt.bfloat16
I16 = mybir.dt.int16

_LAST_EXEC_NS = None        # set when BASS_GNN_TRACE=1
_LAST_RESULTS = None


# ---------------------------------------------------------------------------
# NTFF profiling hook (only used when BASS_GNN_TRACE=1); injects the missing
# antenv.axon_hooks module using ctypes against libaxon_pjrt.so.
# ---------------------------------------------------------------------------
def _install_ntff_hook():
    so = "/opt/axon/libaxon_pjrt.so"
    if "antenv.axon_hooks" in sys.modules or not os.path.exists(so):
        return
    lib = ctypes.CDLL(so)
    if not hasattr(lib, "axon_start_nrt_profile"):
        return
    lib.axon_start_nrt_profile.argtypes = [ctypes.POINTER(ctypes.c_int64), ctypes.c_size_t]
    lib.axon_start_nrt_profile.restype = ctypes.c_int64
    lib.axon_stop_nrt_profile.argtypes = [ctypes.c_char_p]
    lib.axon_stop_nrt_profile.restype = ctypes.c_int64

    @contextlib.contextmanager
    def _hook(output_dir, device_ids):
        import jax

        jax.devices()
        if device_ids:
            ids = (ctypes.c_int64 * len(device_ids))(*device_ids)
            rc = lib.axon_start_nrt_profile(ids, len(device_ids))
        else:
            rc = lib.axon_start_nrt_profile(None, 0)
        if rc != 0:
            raise RuntimeError(f"axon_start_nrt_profile rc={rc}")
        try:
            yield
        finally:
            n = lib.axon_stop_nrt_profile(str(output_dir).encode())
            print(f"profile: {n} ntff file(s) -> {output_dir}", file=sys.stderr)

    mod = types.ModuleType("antenv.axon_hooks")
    mod.get_axon_ntff_profile_hook = lambda: _hook
    mod.set_axon_ntff_profile_hook = lambda h: None
    sys.modules["antenv.axon_hooks"] = mod


def _groups():
    out = []
    b = 0
    while b < NBD:
        nb = min(GB, NBD - b)
        out.append((b, nb))
        b += nb
    return out


# ---------------------------------------------------------------------------
# Host-side edge preprocessing
# ---------------------------------------------------------------------------
def _preprocess(edge_index):
    """Bucket doubled edges by destination block, split lo/hi by sender, and
    build per-core int16 gather-index (wrapped) + local-receiver tiles.

    Returns (idx16_tiles[c], rloc_tiles[c], layout) where layout carries the
    per-block chunk-column ranges and per-group gather-call extents.
    """
    send = np.concatenate([edge_index[0], edge_index[1]]).astype(np.int64)
    recv = np.concatenate([edge_index[1], edge_index[0]]).astype(np.int64)

    blk = recv // P                          # global block id, 0..391
    hi = (send >= SPLIT).astype(np.int64)
    order = np.lexsort((hi, blk))            # by block, lo before hi
    send_s = send[order]
    recv_s = recv[order]
    blk_s = blk[order]
    hi_s = hi[order]

    n_blk_glob = N_PAD // P                  # 392
    counts = np.bincount(blk_s, minlength=n_blk_glob)
    nlo_g = np.bincount(blk_s[hi_s == 0], minlength=n_blk_glob)
    nhi_g = counts - nlo_g
    nlo_cb = nlo_g.reshape(N_CORES, NB)
    nhi_cb = nhi_g.reshape(N_CORES, NB)
    Klo_r = np.maximum(np.ceil(nlo_cb.max(axis=0) / P).astype(np.int64), 1)
    Khi_r = np.ceil(nhi_cb.max(axis=0) / P).astype(np.int64)
    # device block 0 is sacrificial: one all-pad lo chunk, no hi chunks
    Klo = np.concatenate([[1], Klo_r])
    Khi = np.concatenate([[0], Khi_r])

    # column layout: per group, lo chunks of its blocks then hi chunks
    lo_start = np.zeros(NBD, np.int64)
    hi_start = np.zeros(NBD, np.int64)
    g_meta = []
    col = 0
    for b0, nb in _groups():
        g_col0 = col
        for b in range(b0, b0 + nb):
            lo_start[b] = col
            col += Klo[b]
        lo_cols = col - g_col0
        for b in range(b0, b0 + nb):
            hi_start[b] = col
            col += Khi[b]
        hi_cols = col - g_col0 - lo_cols
        g_meta.append((g_col0, lo_cols, hi_cols))
        assert lo_cols * P <= 15000 and hi_cols * P <= 15000, (
            "gather call exceeds Q7 idx scratch"
        )
    TOT = int(col)

    starts = np.concatenate([[0], np.cumsum(counts)])
    # rank of each edge within its (block, stream) segment
    j_all = np.arange(send_s.shape[0]) - starts[blk_s]
    j_seg = np.where(hi_s == 0, j_all, j_all - nlo_g[blk_s])

    b_local = blk_s % NB + 1     # device block index (0 is sacrificial)
    seg_start = np.where(hi_s == 0, lo_start[b_local], hi_start[b_local])
    col_e = seg_start + j_seg // P
    lane_e = j_seg % P
    val_e = np.where(hi_s == 0, send_s, send_s - SPLIT).astype(np.int16)
    rloc_e = (recv_s - (blk_s * P)).astype(np.float32)

    idx16_tiles, rloc_tiles = [], []
    n_wrap = np.arange(TOT * P)
    for c in range(N_CORES):
        lo, hic = starts[c * NB], starts[(c + 1) * NB]
        sl = slice(lo, hic)
        flat = np.zeros(TOT * P, dtype=np.int16)
        flat[col_e[sl] * P + lane_e[sl]] = val_e[sl]
        idx16 = np.zeros((P, TOT * 8), dtype=np.int16)
        for r in range(8):
            idx16[16 * r + (n_wrap % 16), n_wrap // 16] = flat
        rloc_t = np.full((P, TOT), -1.0, dtype=np.float32)
        rloc_t[lane_e[sl], col_e[sl]] = rloc_e[sl]
        idx16_tiles.append(idx16)
        rloc_tiles.append(rloc_t.astype(ml_dtypes.bfloat16))

    layout = (Klo, Khi, lo_start, hi_start, g_meta, TOT)
    return idx16_tiles, rloc_tiles, layout


# ---------------------------------------------------------------------------
# Kernel build
# ---------------------------------------------------------------------------
def _build(layout):
    Klo, Khi, lo_start, hi_start, g_meta, TOT = layout
    nc = bacc.Bacc(
        "TRN2",
        target_bir_lowering=False,
        debug=False,
        num_devices=N_CORES,
        dynamic_dma_scratch_size=49152,
    )

    x = nc.declare_dram_parameter("x", [N_NODES, D_IN], BF16, isOutput=False)
    idx = nc.declare_dram_parameter("idx", [P, TOT * 8], I16, isOutput=False)
    # rl = iota (128 cols) | rloc (TOT cols)
    rl = nc.declare_dram_parameter("rl", [P, P + TOT], BF16, isOutput=False)
    metaf = nc.declare_dram_parameter("metaf", [P, 7], F32, isOutput=False)
    w1 = nc.declare_dram_parameter("w1", [D_IN, D_HID], BF16, isOutput=False)
    w2 = nc.declare_dram_parameter("w2", [D_HID, D_HID], BF16, isOutput=False)
    w3 = nc.declare_dram_parameter("w3", [D_HID, D_OUT], BF16, isOutput=False)
    out = nc.declare_dram_parameter("out", [NPCD, D_OUT], F32, isOutput=True)

    AF = mybir.ActivationFunctionType
    OP = mybir.AluOpType

    with tile.TileContext(nc) as tc:
        sems = [nc.alloc_semaphore(f"gsem{i}") for i in range(NSEM)]
        uses = [0] * NSEM
        sem_rot = [0]  # next sem index

        with (
            tc.tile_pool(name="const", bufs=1) as cpool,
            tc.tile_pool(name="gather", bufs=2) as gpool,
            tc.tile_pool(name="spool", bufs=2) as spool,
            tc.tile_pool(name="agg", bufs=1) as apool,
            tc.tile_pool(name="hid", bufs=7) as hpool,
            tc.tile_pool(name="rows", bufs=5) as rpool,
            tc.tile_pool(name="outp", bufs=2) as opool,
            tc.tile_pool(name="ps1", bufs=2, space="PSUM") as ps1pool,
            tc.tile_pool(name="ps2", bufs=4, space="PSUM") as ps2pool,
            tc.tile_pool(name="psr", bufs=2, space="PSUM") as psrpool,
        ):
            # ---- constants -------------------------------------------------
            idx_sb = cpool.tile([P, TOT * 8], I16)
            nc.sync.dma_start(out=idx_sb[:], in_=idx[:])
            rl_sb = cpool.tile([P, P + TOT], BF16)
            nc.sync.dma_start(out=rl_sb[:], in_=rl[:])
            iota_sb = rl_sb[:, 0:P]
            rloc_sb = rl_sb[:, P : P + TOT]

            metaf_sb = cpool.tile([P, 7], F32)
            nc.sync.dma_start(out=metaf_sb[:], in_=metaf[:])
            b1_ap = metaf_sb[:, 0:2]
            b2_ap = metaf_sb[:, 2:4]
            b3_ap = metaf_sb[:, 4:5]
            lng_ap = metaf_sb[:, 5:6]
            lnb_ap = metaf_sb[:, 6:7]

            w1_sb = cpool.tile([P, D_HID], BF16)
            nc.sync.dma_start(out=w1_sb[:], in_=w1[:])
            w2_sb = cpool.tile([P, 2 * D_HID], BF16)
            nc.sync.dma_start(
                out=w2_sb[:].rearrange("p (h j) -> p h j", h=2),
                in_=w2[:].rearrange("(h p) j -> p h j", p=P),
            )
            w3_sb = cpool.tile([P, 2 * D_OUT], BF16)
            nc.sync.dma_start(
                out=w3_sb[:].rearrange("p (h j) -> p h j", h=2),
                in_=w3[:].rearrange("(h p) j -> p h j", p=P),
            )

            ident_sb = cpool.tile([P, P], F32)
            make_identity(nc, ident_sb[:])
            ones_col = cpool.tile([P, 1], F32)
            nc.vector.memset(ones_col[:], 1.0)
            ones_row = cpool.tile([1, P], F32)
            nc.vector.memset(ones_row[:], 1.0)

            aggB = apool.tile([P, NPCD], BF16)   # [feat, node] for this core

            # Pool-engine touch of idx_sb: gives the gather preps (whose
            # metadata read tile does not gate on the load DMA completion)
            # a properly-waited predecessor in Pool program order
            idx_tok = cpool.tile([P, 8], I16)
            nc.gpsimd.tensor_copy(out=idx_tok[:], in_=idx_sb[:, 0:8])

            # sacrificial warmup gather: the first ext-ISA dma_gather after
            # the IRAM library load misbehaves; absorb it with a dummy call
            warm = cpool.tile([P, P], BF16)
            nc.gpsimd.dma_start(out=warm[:], in_=x[0:P, :])

            # the SWDGE descriptor ring holds ~64 descs per DMA engine and a
            # single prep must fit it whole: cap each call at 7 chunk columns
            # (896 idxs -> 57 descs/DMA incl. sem)
            CALL_COLS = 7

            def gather_call(mt, dst_c0, cols, src_lo, g_col0):
                """Batched dma_gather of `cols` chunk columns, split into
                ring-sized prep+trigger sub-calls. Returns (sem, target)s."""
                waits = []
                done = 0
                while done < cols:
                    cc = min(CALL_COLS, cols - done)
                    s = sem_rot[0]
                    sem_rot[0] = (s + 1) % NSEM
                    c0 = dst_c0 + done
                    nc.gpsimd.dma_gather(
                        out_ap=mt[:, c0 * P : (c0 + cc) * P].rearrange(
                            "p (k j) -> p k j", k=cc
                        ),
                        in_ap=x[0:SPLIT, :] if src_lo else x[SPLIT:N_NODES, :],
                        idxs_ap=idx_sb[:, (g_col0 + done) * 8 : (g_col0 + done + cc) * 8],
                        num_idxs=cc * P,
                        num_idxs_reg=cc * P,
                        elem_size=D_IN,
                        prepare_only=True,
                        sem=sems[s],
                    )
                    nc.gpsimd.trigger_dma(count=None)
                    uses[s] += 1
                    waits.append((s, 16 * uses[s]))
                    done += cc
                return waits

            # ---- phase 2: transposed MLP + LayerNorm on a node group -------
            def phase2_group(g0, ng):
                rhs_agg = aggB[:, g0 : g0 + ng]
                h1 = []
                for jh in range(2):
                    p1 = ps2pool.tile([P, ng], F32, tag="p2")
                    nc.tensor.matmul(
                        out=p1[:], lhsT=w1_sb[:, jh * P : (jh + 1) * P],
                        rhs=rhs_agg, start=True, stop=True,
                    )
                    t = hpool.tile([P, ng], BF16, tag="h")
                    nc.scalar.activation(t[:], p1[:], AF.Gelu, bias=b1_ap[:, jh : jh + 1])
                    h1.append(t)
                h2 = []
                for kh in range(2):
                    p2 = ps2pool.tile([P, ng], F32, tag="p2")
                    for jh in range(2):
                        nc.tensor.matmul(
                            out=p2[:],
                            lhsT=w2_sb[:, jh * D_HID + kh * P : jh * D_HID + (kh + 1) * P],
                            rhs=h1[jh][:], start=(jh == 0), stop=(jh == 1),
                        )
                    t = hpool.tile([P, ng], BF16, tag="h")
                    nc.scalar.activation(t[:], p2[:], AF.Gelu, bias=b2_ap[:, kh : kh + 1])
                    h2.append(t)
                p3 = ps2pool.tile([P, ng], F32, tag="p2")
                for kh in range(2):
                    nc.tensor.matmul(
                        out=p3[:], lhsT=w3_sb[:, kh * D_OUT : (kh + 1) * D_OUT],
                        rhs=h2[kh][:], start=(kh == 0), stop=(kh == 1),
                    )
                h3 = hpool.tile([P, ng], F32, tag="hf")
                nc.scalar.activation(h3[:], p3[:], AF.Identity, bias=b3_ap)
                sq = hpool.tile([P, ng], F32, tag="hf")
                nc.scalar.activation(sq[:], h3[:], AF.Square)

                mu_ps = psrpool.tile([1, ng], F32, tag="pr")
                nc.tensor.matmul(out=mu_ps[:], lhsT=ones_col[:], rhs=h3[:], start=True, stop=True)
                s2_ps = psrpool.tile([1, ng], F32, tag="pr")
                nc.tensor.matmul(out=s2_ps[:], lhsT=ones_col[:], rhs=sq[:], start=True, stop=True)

                m_row = rpool.tile([1, ng], F32, tag="r")
                nc.vector.tensor_scalar_mul(m_row[:], mu_ps[:], 1.0 / P)
                q_row = rpool.tile([1, ng], F32, tag="r")
                nc.vector.tensor_tensor(out=q_row[:], in0=m_row[:], in1=m_row[:], op=OP.mult)
                v_row = rpool.tile([1, ng], F32, tag="r")
                nc.vector.tensor_scalar_mul(v_row[:], s2_ps[:], 1.0 / P)
                nc.vector.tensor_tensor(out=v_row[:], in0=v_row[:], in1=q_row[:], op=OP.subtract)
                nc.vector.tensor_scalar_add(v_row[:], v_row[:], 1e-5)
                sdev = rpool.tile([1, ng], F32, tag="r")
                nc.scalar.activation(sdev[:], v_row[:], AF.Sqrt)
                inv_row = rpool.tile([1, ng], F32, tag="r")
                with nc.allow_low_precision("matching jax rsqrt f32"):
                    nc.vector.reciprocal(inv_row[:], sdev[:])
                minv_row = rpool.tile([1, ng], F32, tag="r")
                nc.vector.tensor_tensor(out=minv_row[:], in0=m_row[:], in1=inv_row[:], op=OP.mult)

                inv_ps = ps2pool.tile([P, ng], F32, tag="p2")
                nc.tensor.matmul(out=inv_ps[:], lhsT=ones_row[:], rhs=inv_row[:], start=True, stop=True)
                minv_ps = ps2pool.tile([P, ng], F32, tag="p2")
                nc.tensor.matmul(out=minv_ps[:], lhsT=ones_row[:], rhs=minv_row[:], start=True, stop=True)

                t1 = hpool.tile([P, ng], F32, tag="hf")
                nc.vector.tensor_tensor(out=t1[:], in0=h3[:], in1=inv_ps[:], op=OP.mult)
                t2 = hpool.tile([P, ng], F32, tag="hf")
                nc.vector.tensor_tensor(out=t2[:], in0=t1[:], in1=minv_ps[:], op=OP.subtract)
                oT = hpool.tile([P, ng], F32, tag="hf")
                nc.vector.tensor_scalar(
                    out=oT[:], in0=t2[:], scalar1=lng_ap, scalar2=lnb_ap,
                    op0=OP.mult, op1=OP.add,
                )

                for t in range(ng // P):
                    trp = ps2pool.tile([P, P], F32, tag="p2")
                    nc.tensor.transpose(out=trp[:], in_=oT[:, t * P : (t + 1) * P], identity=ident_sb[:])
                    ot = opool.tile([P, P], F32, tag="o")
                    nc.scalar.copy(out=ot[:], in_=trp[:])
                    r0 = g0 + t * P
                    nc.sync.dma_start(out=out[r0 : r0 + P, :], in_=ot[:])

            # ---- main loop: gather group -> blocks -> MLP group ------------
            for gi, (b0, nb) in enumerate(_groups()):
                g_col0, lo_cols, hi_cols = g_meta[gi]
                totg = lo_cols + hi_cols
                mt = gpool.tile([P, totg * P], BF16, tag="m")
                w_lo = gather_call(mt, 0, lo_cols, True, g_col0)
                w_hi = gather_call(mt, lo_cols, hi_cols, False, g_col0 + lo_cols)

                # gather data is consumed by PE via the S-build's engine:
                # explicit DVE waits on the DMA completion sems before the
                # one-hot build (tile only orders against the prep); the PE
                # matmuls read S, so they transitively run after the waits.
                # dedupe by sem, keeping the highest target
                needed = {}
                for s_i, tgt in w_lo + w_hi:
                    needed[s_i] = max(needed.get(s_i, 0), tgt)
                w_insts = [
                    nc.vector.wait_ge(sems[s_i], tgt) for s_i, tgt in needed.items()
                ]

                # batched one-hot build over the whole group's columns; pinned
                # after the waits so the PE matmuls (which wait on this build)
                # transitively wait for the gather DMA completions
                s = spool.tile([P, totg * P], BF16, tag="s")
                s_call = nc.vector.tensor_tensor(
                    out=s[:].rearrange("p (k j) -> p k j", k=totg),
                    in0=iota_sb.unsqueeze(1).broadcast_to([P, totg, P]),
                    in1=rloc_sb[:, g_col0 : g_col0 + totg].unsqueeze(2).broadcast_to(
                        [P, totg, P]
                    ),
                    op=OP.is_equal,
                )
                dep_set = InstructionNameOrderedSet()
                for w in w_insts:
                    dep_set.add(w.ins.name)
                if len(dep_set):
                    s_call.ins.add_nosync_dependencies_from(dep_set)

                for b in range(b0, b0 + nb):
                    cols = []
                    cols.extend(range(int(lo_start[b]) - g_col0, int(lo_start[b]) - g_col0 + int(Klo[b])))
                    cols.extend(range(int(hi_start[b]) - g_col0, int(hi_start[b]) - g_col0 + int(Khi[b])))
                    ps = ps1pool.tile([P, P], F32, tag="p1")
                    for ki, k in enumerate(cols):
                        nc.tensor.matmul(
                            out=ps[:],
                            lhsT=mt[:, k * D_IN : (k + 1) * D_IN],
                            rhs=s[:, k * P : (k + 1) * P],
                            start=(ki == 0),
                            stop=(ki == len(cols) - 1),
                        )
                    nc.scalar.copy(out=aggB[:, b * P : (b + 1) * P], in_=ps[:])

                phase2_group(b0 * P, nb * P)
    nc.compile()
    return nc


# ---------------------------------------------------------------------------
# Public entry point
# ---------------------------------------------------------------------------
def kernel(x, edge_index, W1, b1, W2, b2, W3, b3, ln_g, ln_b):
    global _LAST_EXEC_NS
    x_bf = np.ascontiguousarray(
        np.asarray(x, dtype=np.float32).astype(ml_dtypes.bfloat16)
    )
    edge_index = np.asarray(edge_index)

    idx16_tiles, rloc_tiles, layout = _preprocess(edge_index)
    TOT = layout[-1]

    iota = np.tile(np.arange(P, dtype=np.float32), (P, 1)).astype(ml_dtypes.bfloat16)
    b1_2 = np.asarray(b1, np.float32).reshape(2, P).T          # [128, 2]
    b2_2 = np.asarray(b2, np.float32).reshape(2, P).T
    b3_1 = np.asarray(b3, np.float32).reshape(1, P).T          # [128, 1]
    g_1 = np.asarray(ln_g, np.float32).reshape(1, P).T
    lb_1 = np.asarray(ln_b, np.float32).reshape(1, P).T
    metaf = np.ascontiguousarray(
        np.concatenate([b1_2, b2_2, b3_1, g_1, lb_1], axis=1).astype(np.float32)
    )

    w1_bf = np.ascontiguousarray(np.asarray(W1, np.float32).astype(ml_dtypes.bfloat16))
    w2_bf = np.ascontiguousarray(np.asarray(W2, np.float32).astype(ml_dtypes.bfloat16))
    w3_bf = np.ascontiguousarray(np.asarray(W3, np.float32).astype(ml_dtypes.bfloat16))

    in_maps = []
    for c in range(N_CORES):
        rl_c = np.ascontiguousarray(np.concatenate([iota, rloc_tiles[c]], axis=1))
        in_maps.append(
            {
                "x": x_bf,
                "idx": idx16_tiles[c],
                "rl": rl_c,
                "metaf": metaf,
                "w1": w1_bf,
                "w2": w2_bf,
                "w3": w3_bf,
            }
        )

    nc = _build(layout)

    trace = os.environ.get("BASS_GNN_TRACE", "0") == "1"
    if trace:
        _install_ntff_hook()
    r = run_bass_kernel_spmd(nc, in_maps, list(range(N_CORES)), trace=trace)
    _LAST_EXEC_NS = r.exec_time_ns

    global _LAST_RESULTS
    _LAST_RESULTS = r.results
    full = np.concatenate(
        [r.results[c]["out"][P:NPCD] for c in range(N_CORES)], axis=0
    )
    return np.ascontiguousarray(full[:N_NODES].astype(np.float32))


# revision 21
# speedup vs baseline: 1.0104x; 1.0048x over previous
"""GNN message passing (nn_NodeToNode) on 8 trn2 NeuronCores via Bass/Tile.

Algorithm (per core, SPMD):
  - Nodes are range-sharded: core c owns nodes [c*6272, (c+1)*6272) (50176 total,
    padded; host slices output back to 50000).
  - Host sorts the doubled edge list by receiver and buckets edges into the
    owner core's 49 node-blocks of 128. Within each block bucket, edges are
    split into a lo stream (sender < 32768) and a hi stream (sender >= 32768)
    because dma_gather indices are int16. Each stream is padded to whole
    128-edge chunks (pad: sender-slot 0, rloc=-1).
  - Phase 1 on device, per 4-block group: TWO batched SWDGE dma_gather calls
    (prepare_only + trigger_dma; lo chunks from x[0:32768], hi chunks from
    x[32768:], indices int16 wrapped [n%16, n//16] and replicated over the 8
    Q7 replica partition groups) fetch all sender rows of bf16 x; ONE batched
    DVE is_equal per stream builds the one-hot S[e, col, n] = (iota[n] ==
    rloc[e, col]); per block, bf16 matmuls accumulate aggT[f, n] += M^T . S
    in PSUM over the block's lo+hi chunk columns. rloc=-1 padding zeroes the
    S rows, masking pad/garbage lanes. PE waits on the gather DMA-completion
    semaphores explicitly (tile does not wait for prepare_only DMA data).
    Batching ~70 chunks per gather call amortizes the ~1us fixed SWDGE cost
    per DMA instruction that dominated the unbatched version.
  - Phase 2 on device (transposed layout, per 512-node group, emitted right
    after its 4 phase-1 blocks for overlap): 3-layer MLP with per-partition
    biases on ACT (exact-erf GELU) with bf16 matmul inputs, LayerNorm over
    the feature (=partition) axis in fp32 via ones-matmul stats +
    replicate-matmul broadcast, then PE transpose back to [node, feat].
"""
import os
import sys
import types
import contextlib
import ctypes

import numpy as np
import ml_dtypes

import concourse.bacc as bacc
import concourse.mybir as mybir
import concourse.tile as tile
from concourse.instruction_name_ordered_set import InstructionNameOrderedSet
from concourse.bass_utils import run_bass_kernel_spmd
from concourse.masks import make_identity

P = 128
N_NODES = 50000
SPLIT = 32768               # lo/hi sender split (int16 index limit)
D_IN = 128
D_HID = 256
D_OUT = 128
N_CORES = 8
NB = 49                     # real node blocks per core
NBD = 50                    # device blocks (block 0 is a sacrificial pad block)
NPC = NB * P                # real nodes per core (6272), 8*6272 = 50176 >= 50000
NPCD = NBD * P              # device rows per core (6400)
N_PAD = N_CORES * NPC
GB = 4                      # blocks per gather/MLP group
NSEM = 8                    # rotating gather-completion semaphores

F32 = mybir.dt.float32
BF16 = mybir.dt.bfloat16
I16 = mybir.dt.int16

_LAST_EXEC_NS = None        # set when BASS_GNN_TRACE=1
_LAST_RESULTS = None


# ---------------------------------------------------------------------------
# NTFF profiling hook (only used when BASS_GNN_TRACE=1); injects the missing
# antenv.axon_hooks module using ctypes against libaxon_pjrt.so.
# ---------------------------------------------------------------------------
def _install_ntff_hook():
    so = "/opt/axon/libaxon_pjrt.so"
    if "antenv.axon_hooks" in sys.modules or not os.path.exists(so):
        return
    lib = ctypes.CDLL(so)
    if not hasattr(lib, "axon_start_nrt_profile"):
        return
    lib.axon_start_nrt_profile.argtypes = [ctypes.POINTER(ctypes.c_int64), ctypes.c_size_t]
    lib.axon_start_nrt_profile.restype = ctypes.c_int64
    lib.axon_stop_nrt_profile.argtypes = [ctypes.c_char_p]
    lib.axon_stop_nrt_profile.restype = ctypes.c_int64

    @contextlib.contextmanager
    def _hook(output_dir, device_ids):
        import jax

        jax.devices()
        if device_ids:
            ids = (ctypes.c_int64 * len(device_ids))(*device_ids)
            rc = lib.axon_start_nrt_profile(ids, len(device_ids))
        else:
            rc = lib.axon_start_nrt_profile(None, 0)
        if rc != 0:
            raise RuntimeError(f"axon_start_nrt_profile rc={rc}")
        try:
            yield
        finally:
            n = lib.axon_stop_nrt_profile(str(output_dir).encode())
            print(f"profile: {n} ntff file(s) -> {output_dir}", file=sys.stderr)

    mod = types.ModuleType("antenv.axon_hooks")
    mod.get_axon_ntff_profile_hook = lambda: _hook
    mod.set_axon_ntff_profile_hook = lambda h: None
    sys.modules["antenv.axon_hooks"] = mod


def _groups():
    out = []
    b = 0
    while b < NBD:
        nb = min(GB, NBD - b)
        out.append((b, nb))
        b += nb
    return out


# ---------------------------------------------------------------------------
# Host-side edge preprocessing
# ---------------------------------------------------------------------------
def _preprocess(edge_index):
    """Bucket doubled edges by destination block, split lo/hi by sender, and
    build per-core int16 gather-index (wrapped) + local-receiver tiles.

    Returns (idx16_tiles[c], rloc_tiles[c], layout) where layout carries the
    per-block chunk-column ranges and per-group gather-call extents.
    """
    send = np.concatenate([edge_index[0], edge_index[1]]).astype(np.int64)
    recv = np.concatenate([edge_index[1], edge_index[0]]).astype(np.int64)

    blk = recv // P                          # global block id, 0..391
    hi = (send >= SPLIT).astype(np.int64)
    order = np.lexsort((hi, blk))            # by block, lo before hi
    send_s = send[order]
    recv_s = recv[order]
    blk_s = blk[order]
    hi_s = hi[order]

    n_blk_glob = N_PAD // P                  # 392
    counts = np.bincount(blk_s, minlength=n_blk_glob)
    nlo_g = np.bincount(blk_s[hi_s == 0], minlength=n_blk_glob)
    nhi_g = counts - nlo_g
    nlo_cb = nlo_g.reshape(N_CORES, NB)
    nhi_cb = nhi_g.reshape(N_CORES, NB)
    Klo_r = np.maximum(np.ceil(nlo_cb.max(axis=0) / P).astype(np.int64), 1)
    Khi_r = np.ceil(nhi_cb.max(axis=0) / P).astype(np.int64)
    # device block 0 is sacrificial: one all-pad lo chunk, no hi chunks
    Klo = np.concatenate([[1], Klo_r])
    Khi = np.concatenate([[0], Khi_r])

    # column layout: per group, lo chunks of its blocks then hi chunks
    lo_start = np.zeros(NBD, np.int64)
    hi_start = np.zeros(NBD, np.int64)
    g_meta = []
    col = 0
    for b0, nb in _groups():
        g_col0 = col
        for b in range(b0, b0 + nb):
            lo_start[b] = col
            col += Klo[b]
        lo_cols = col - g_col0
        for b in range(b0, b0 + nb):
            hi_start[b] = col
            col += Khi[b]
        hi_cols = col - g_col0 - lo_cols
        g_meta.append((g_col0, lo_cols, hi_cols))
        assert lo_cols * P <= 15000 and hi_cols * P <= 15000, (
            "gather call exceeds Q7 idx scratch"
        )
    TOT = int(col)

    starts = np.concatenate([[0], np.cumsum(counts)])
    # rank of each edge within its (block, stream) segment
    j_all = np.arange(send_s.shape[0]) - starts[blk_s]
    j_seg = np.where(hi_s == 0, j_all, j_all - nlo_g[blk_s])

    b_local = blk_s % NB + 1     # device block index (0 is sacrificial)
    seg_start = np.where(hi_s == 0, lo_start[b_local], hi_start[b_local])
    col_e = seg_start + j_seg // P
    lane_e = j_seg % P
    val_e = np.where(hi_s == 0, send_s, send_s - SPLIT).astype(np.int16)
    rloc_e = (recv_s - (blk_s * P)).astype(np.float32)

    idx16_tiles, rloc_tiles = [], []
    n_wrap = np.arange(TOT * P)
    for c in range(N_CORES):
        lo, hic = starts[c * NB], starts[(c + 1) * NB]
        sl = slice(lo, hic)
        flat = np.zeros(TOT * P, dtype=np.int16)
        flat[col_e[sl] * P + lane_e[sl]] = val_e[sl]
        idx16 = np.zeros((P, TOT * 8), dtype=np.int16)
        for r in range(8):
            idx16[16 * r + (n_wrap % 16), n_wrap // 16] = flat
        rloc_t = np.full((P, TOT), -1.0, dtype=np.float32)
        rloc_t[lane_e[sl], col_e[sl]] = rloc_e[sl]
        idx16_tiles.append(idx16)
        rloc_tiles.append(rloc_t.astype(ml_dtypes.bfloat16))

    layout = (Klo, Khi, lo_start, hi_start, g_meta, TOT)
    return idx16_tiles, rloc_tiles, layout


# ---------------------------------------------------------------------------
# Kernel build
# ---------------------------------------------------------------------------
def _build(layout):
    Klo, Khi, lo_start, hi_start, g_meta, TOT = layout
    nc = bacc.Bacc(
        "TRN2",
        target_bir_lowering=False,
        debug=False,
        num_devices=N_CORES,
        num_swdge_queues=4,
    )

    x = nc.declare_dram_parameter("x", [N_NODES, D_IN], BF16, isOutput=False)
    idx = nc.declare_dram_parameter("idx", [P, TOT * 8], I16, isOutput=False)
    # rl = iota (128 cols) | rloc (TOT cols)
    rl = nc.declare_dram_parameter("rl", [P, P + TOT], BF16, isOutput=False)
    metaf = nc.declare_dram_parameter("metaf", [P, 7], F32, isOutput=False)
    w1 = nc.declare_dram_parameter("w1", [D_IN, D_HID], BF16, isOutput=False)
    w2 = nc.declare_dram_parameter("w2", [D_HID, D_HID], BF16, isOutput=False)
    w3 = nc.declare_dram_parameter("w3", [D_HID, D_OUT], BF16, isOutput=False)
    out = nc.declare_dram_parameter("out", [NPCD, D_OUT], F32, isOutput=True)

    AF = mybir.ActivationFunctionType
    OP = mybir.AluOpType

    with tile.TileContext(nc) as tc:
        sems = [nc.alloc_semaphore(f"gsem{i}") for i in range(NSEM)]
        uses = [0] * NSEM
        sem_rot = [0]  # next sem index

        with (
            tc.tile_pool(name="const", bufs=1) as cpool,
            tc.tile_pool(name="gather", bufs=2) as gpool,
            tc.tile_pool(name="spool", bufs=2) as spool,
            tc.tile_pool(name="agg", bufs=1) as apool,
            tc.tile_pool(name="hid", bufs=7) as hpool,
            tc.tile_pool(name="rows", bufs=5) as rpool,
            tc.tile_pool(name="outp", bufs=2) as opool,
            tc.tile_pool(name="ps1", bufs=2, space="PSUM") as ps1pool,
            tc.tile_pool(name="ps2", bufs=4, space="PSUM") as ps2pool,
            tc.tile_pool(name="psr", bufs=2, space="PSUM") as psrpool,
        ):
            # ---- constants -------------------------------------------------
            idx_sb = cpool.tile([P, TOT * 8], I16)
            nc.sync.dma_start(out=idx_sb[:], in_=idx[:])
            rl_sb = cpool.tile([P, P + TOT], BF16)
            nc.sync.dma_start(out=rl_sb[:], in_=rl[:])
            iota_sb = rl_sb[:, 0:P]
            rloc_sb = rl_sb[:, P : P + TOT]

            metaf_sb = cpool.tile([P, 7], F32)
            nc.sync.dma_start(out=metaf_sb[:], in_=metaf[:])
            b1_ap = metaf_sb[:, 0:2]
            b2_ap = metaf_sb[:, 2:4]
            b3_ap = metaf_sb[:, 4:5]
            lng_ap = metaf_sb[:, 5:6]
            lnb_ap = metaf_sb[:, 6:7]

            w1_sb = cpool.tile([P, D_HID], BF16)
            nc.sync.dma_start(out=w1_sb[:], in_=w1[:])
            w2_sb = cpool.tile([P, 2 * D_HID], BF16)
            nc.sync.dma_start(
                out=w2_sb[:].rearrange("p (h j) -> p h j", h=2),
                in_=w2[:].rearrange("(h p) j -> p h j", p=P),
            )
            w3_sb = cpool.tile([P, 2 * D_OUT], BF16)
            nc.sync.dma_start(
                out=w3_sb[:].rearrange("p (h j) -> p h j", h=2),
                in_=w3[:].rearrange("(h p) j -> p h j", p=P),
            )

            ident_sb = cpool.tile([P, P], F32)
            make_identity(nc, ident_sb[:])
            ones_col = cpool.tile([P, 1], F32)
            nc.vector.memset(ones_col[:], 1.0)
            ones_row = cpool.tile([1, P], F32)
            nc.vector.memset(ones_row[:], 1.0)

            aggB = apool.tile([P, NPCD], BF16)   # [feat, node] for this core

            # Pool-engine touch of idx_sb: gives the gather preps (whose
            # metadata read tile does not gate on the load DMA completion)
            # a properly-waited predecessor in Pool program order
            idx_tok = cpool.tile([P, 8], I16)
            nc.gpsimd.tensor_copy(out=idx_tok[:], in_=idx_sb[:, 0:8])

            # sacrificial warmup gather: the first ext-ISA dma_gather after
            # the IRAM library load misbehaves; absorb it with a dummy call
            warm = cpool.tile([P, P], BF16)
            nc.gpsimd.dma_start(out=warm[:], in_=x[0:P, :])

            # the SWDGE descriptor ring holds ~64 descs per DMA engine and a
            # single prep must fit it whole: cap each call at 7 chunk columns
            # (896 idxs -> 57 descs/DMA incl. sem)
            CALL_COLS = 7

            def gather_call(mt, dst_c0, cols, src_lo, g_col0):
                """Batched dma_gather of `cols` chunk columns, split into
                ring-sized prep+trigger sub-calls. Returns (sem, target)s."""
                waits = []
                done = 0
                while done < cols:
                    cc = min(CALL_COLS, cols - done)
                    s = sem_rot[0]
                    sem_rot[0] = (s + 1) % NSEM
                    qn = s % 4
                    c0 = dst_c0 + done
                    nc.gpsimd.dma_gather(
                        out_ap=mt[:, c0 * P : (c0 + cc) * P].rearrange(
                            "p (k j) -> p k j", k=cc
                        ),
                        in_ap=x[0:SPLIT, :] if src_lo else x[SPLIT:N_NODES, :],
                        idxs_ap=idx_sb[:, (g_col0 + done) * 8 : (g_col0 + done + cc) * 8],
                        num_idxs=cc * P,
                        num_idxs_reg=cc * P,
                        elem_size=D_IN,
                        prepare_only=True,
                        sem=sems[s],
                        queue_num=qn,
                    )
                    nc.gpsimd.trigger_dma(count=None, queue_num=qn)
                    uses[s] += 1
                    waits.append((s, 16 * uses[s]))
                    done += cc
                return waits

            # ---- phase 2: transposed MLP + LayerNorm on a node group -------
            def phase2_group(g0, ng):
                rhs_agg = aggB[:, g0 : g0 + ng]
                h1 = []
                for jh in range(2):
                    p1 = ps2pool.tile([P, ng], F32, tag="p2")
                    nc.tensor.matmul(
                        out=p1[:], lhsT=w1_sb[:, jh * P : (jh + 1) * P],
                        rhs=rhs_agg, start=True, stop=True,
                    )
                    t = hpool.tile([P, ng], BF16, tag="h")
                    nc.scalar.activation(t[:], p1[:], AF.Gelu, bias=b1_ap[:, jh : jh + 1])
                    h1.append(t)
                h2 = []
                for kh in range(2):
                    p2 = ps2pool.tile([P, ng], F32, tag="p2")
                    for jh in range(2):
                        nc.tensor.matmul(
                            out=p2[:],
                            lhsT=w2_sb[:, jh * D_HID + kh * P : jh * D_HID + (kh + 1) * P],
                            rhs=h1[jh][:], start=(jh == 0), stop=(jh == 1),
                        )
                    t = hpool.tile([P, ng], BF16, tag="h")
                    nc.scalar.activation(t[:], p2[:], AF.Gelu, bias=b2_ap[:, kh : kh + 1])
                    h2.append(t)
                p3 = ps2pool.tile([P, ng], F32, tag="p2")
                for kh in range(2):
                    nc.tensor.matmul(
                        out=p3[:], lhsT=w3_sb[:, kh * D_OUT : (kh + 1) * D_OUT],
                        rhs=h2[kh][:], start=(kh == 0), stop=(kh == 1),
                    )
                h3 = hpool.tile([P, ng], F32, tag="hf")
                nc.scalar.activation(h3[:], p3[:], AF.Identity, bias=b3_ap)
                sq = hpool.tile([P, ng], F32, tag="hf")
                nc.scalar.activation(sq[:], h3[:], AF.Square)

                mu_ps = psrpool.tile([1, ng], F32, tag="pr")
                nc.tensor.matmul(out=mu_ps[:], lhsT=ones_col[:], rhs=h3[:], start=True, stop=True)
                s2_ps = psrpool.tile([1, ng], F32, tag="pr")
                nc.tensor.matmul(out=s2_ps[:], lhsT=ones_col[:], rhs=sq[:], start=True, stop=True)

                m_row = rpool.tile([1, ng], F32, tag="r")
                nc.vector.tensor_scalar_mul(m_row[:], mu_ps[:], 1.0 / P)
                q_row = rpool.tile([1, ng], F32, tag="r")
                nc.vector.tensor_tensor(out=q_row[:], in0=m_row[:], in1=m_row[:], op=OP.mult)
                v_row = rpool.tile([1, ng], F32, tag="r")
                nc.vector.tensor_scalar_mul(v_row[:], s2_ps[:], 1.0 / P)
                nc.vector.tensor_tensor(out=v_row[:], in0=v_row[:], in1=q_row[:], op=OP.subtract)
                nc.vector.tensor_scalar_add(v_row[:], v_row[:], 1e-5)
                sdev = rpool.tile([1, ng], F32, tag="r")
                nc.scalar.activation(sdev[:], v_row[:], AF.Sqrt)
                inv_row = rpool.tile([1, ng], F32, tag="r")
                with nc.allow_low_precision("matching jax rsqrt f32"):
                    nc.vector.reciprocal(inv_row[:], sdev[:])
                minv_row = rpool.tile([1, ng], F32, tag="r")
                nc.vector.tensor_tensor(out=minv_row[:], in0=m_row[:], in1=inv_row[:], op=OP.mult)

                inv_ps = ps2pool.tile([P, ng], F32, tag="p2")
                nc.tensor.matmul(out=inv_ps[:], lhsT=ones_row[:], rhs=inv_row[:], start=True, stop=True)
                minv_ps = ps2pool.tile([P, ng], F32, tag="p2")
                nc.tensor.matmul(out=minv_ps[:], lhsT=ones_row[:], rhs=minv_row[:], start=True, stop=True)

                t1 = hpool.tile([P, ng], F32, tag="hf")
                nc.vector.tensor_tensor(out=t1[:], in0=h3[:], in1=inv_ps[:], op=OP.mult)
                t2 = hpool.tile([P, ng], F32, tag="hf")
                nc.vector.tensor_tensor(out=t2[:], in0=t1[:], in1=minv_ps[:], op=OP.subtract)
                oT = hpool.tile([P, ng], F32, tag="hf")
                nc.vector.tensor_scalar(
                    out=oT[:], in0=t2[:], scalar1=lng_ap, scalar2=lnb_ap,
                    op0=OP.mult, op1=OP.add,
                )

                for t in range(ng // P):
                    trp = ps2pool.tile([P, P], F32, tag="p2")
                    nc.tensor.transpose(out=trp[:], in_=oT[:, t * P : (t + 1) * P], identity=ident_sb[:])
                    ot = opool.tile([P, P], F32, tag="o")
                    nc.scalar.copy(out=ot[:], in_=trp[:])
                    r0 = g0 + t * P
                    nc.sync.dma_start(out=out[r0 : r0 + P, :], in_=ot[:])

            # ---- main loop: gather group -> blocks -> MLP group ------------
            for gi, (b0, nb) in enumerate(_groups()):
                g_col0, lo_cols, hi_cols = g_meta[gi]
                totg = lo_cols + hi_cols
                mt = gpool.tile([P, totg * P], BF16, tag="m")
                w_lo = gather_call(mt, 0, lo_cols, True, g_col0)
                w_hi = gather_call(mt, lo_cols, hi_cols, False, g_col0 + lo_cols)

                # gather data is consumed by PE via the S-build's engine:
                # explicit DVE waits on the DMA completion sems before the
                # one-hot build (tile only orders against the prep); the PE
                # matmuls read S, so they transitively run after the waits.
                # dedupe by sem, keeping the highest target
                needed = {}
                for s_i, tgt in w_lo + w_hi:
                    needed[s_i] = max(needed.get(s_i, 0), tgt)
                w_insts = [
                    nc.vector.wait_ge(sems[s_i], tgt) for s_i, tgt in needed.items()
                ]

                # batched one-hot build over the whole group's columns; pinned
                # after the waits so the PE matmuls (which wait on this build)
                # transitively wait for the gather DMA completions
                s = spool.tile([P, totg * P], BF16, tag="s")
                s_call = nc.vector.tensor_tensor(
                    out=s[:].rearrange("p (k j) -> p k j", k=totg),
                    in0=iota_sb.unsqueeze(1).broadcast_to([P, totg, P]),
                    in1=rloc_sb[:, g_col0 : g_col0 + totg].unsqueeze(2).broadcast_to(
                        [P, totg, P]
                    ),
                    op=OP.is_equal,
                )
                dep_set = InstructionNameOrderedSet()
                for w in w_insts:
                    dep_set.add(w.ins.name)
                if len(dep_set):
                    s_call.ins.add_nosync_dependencies_from(dep_set)

                for b in range(b0, b0 + nb):
                    cols = []
                    cols.extend(range(int(lo_start[b]) - g_col0, int(lo_start[b]) - g_col0 + int(Klo[b])))
                    cols.extend(range(int(hi_start[b]) - g_col0, int(hi_start[b]) - g_col0 + int(Khi[b])))
                    ps = ps1pool.tile([P, P], F32, tag="p1")
                    for ki, k in enumerate(cols):
                        nc.tensor.matmul(
                            out=ps[:],
                            lhsT=mt[:, k * D_IN : (k + 1) * D_IN],
                            rhs=s[:, k * P : (k + 1) * P],
                            start=(ki == 0),
                            stop=(ki == len(cols) - 1),
                        )
                    nc.scalar.copy(out=aggB[:, b * P : (b + 1) * P], in_=ps[:])

                phase2_group(b0 * P, nb * P)
    nc.compile()
    return nc


# ---------------------------------------------------------------------------
# Public entry point
# ---------------------------------------------------------------------------
def kernel(x, edge_index, W1, b1, W2, b2, W3, b3, ln_g, ln_b):
    global _LAST_EXEC_NS
    x_bf = np.ascontiguousarray(
        np.asarray(x, dtype=np.float32).astype(ml_dtypes.bfloat16)
    )
    edge_index = np.asarray(edge_index)

    idx16_tiles, rloc_tiles, layout = _preprocess(edge_index)
    TOT = layout[-1]

    iota = np.tile(np.arange(P, dtype=np.float32), (P, 1)).astype(ml_dtypes.bfloat16)
    b1_2 = np.asarray(b1, np.float32).reshape(2, P).T          # [128, 2]
    b2_2 = np.asarray(b2, np.float32).reshape(2, P).T
    b3_1 = np.asarray(b3, np.float32).reshape(1, P).T          # [128, 1]
    g_1 = np.asarray(ln_g, np.float32).reshape(1, P).T
    lb_1 = np.asarray(ln_b, np.float32).reshape(1, P).T
    metaf = np.ascontiguousarray(
        np.concatenate([b1_2, b2_2, b3_1, g_1, lb_1], axis=1).astype(np.float32)
    )

    w1_bf = np.ascontiguousarray(np.asarray(W1, np.float32).astype(ml_dtypes.bfloat16))
    w2_bf = np.ascontiguousarray(np.asarray(W2, np.float32).astype(ml_dtypes.bfloat16))
    w3_bf = np.ascontiguousarray(np.asarray(W3, np.float32).astype(ml_dtypes.bfloat16))

    in_maps = []
    for c in range(N_CORES):
        rl_c = np.ascontiguousarray(np.concatenate([iota, rloc_tiles[c]], axis=1))
        in_maps.append(
            {
                "x": x_bf,
                "idx": idx16_tiles[c],
                "rl": rl_c,
                "metaf": metaf,
                "w1": w1_bf,
                "w2": w2_bf,
                "w3": w3_bf,
            }
        )

    nc = _build(layout)

    trace = os.environ.get("BASS_GNN_TRACE", "0") == "1"
    if trace:
        _install_ntff_hook()
    r = run_bass_kernel_spmd(nc, in_maps, list(range(N_CORES)), trace=trace)
    _LAST_EXEC_NS = r.exec_time_ns

    global _LAST_RESULTS
    _LAST_RESULTS = r.results
    full = np.concatenate(
        [r.results[c]["out"][P:NPCD] for c in range(N_CORES)], axis=0
    )
    return np.ascontiguousarray(full[:N_NODES].astype(np.float32))


# revision 23
# speedup vs baseline: 1.1583x; 1.1464x over previous
"""GNN message passing (nn_NodeToNode) on 8 trn2 NeuronCores via Bass/Tile.

Algorithm (per core, SPMD):
  - Nodes are range-sharded: core c owns nodes [c*6272, (c+1)*6272) (50176 total,
    padded; host slices output back to 50000).
  - Host sorts the doubled edge list by receiver and buckets edges into the
    owner core's 49 node-blocks of 128. Per (core, block) the edge list is
    padded to whole 128-edge chunks (pad: sender=0, rloc=-1).
  - Phase 1 on device: for each chunk, gather 128 sender rows of x (512B each)
    via vector-indirect DMA (one descriptor per row), build the one-hot
    S[e, n] = (iota[n] == rloc[e]) on DVE, and accumulate
    aggT[f, n] += M[e, f]^T . S[e, n] into PSUM over the block's chunks.
    rloc=-1 padding makes S rows zero, masking pad/garbage lanes.
  - Phase 2 on device (transposed layout, per 512-node group): 3-layer MLP with
    per-partition biases on ACT (exact-erf GELU), LayerNorm over the feature
    (=partition) axis via ones-matmul stats + replicate-matmul broadcast,
    then PE transpose back to [node, feat] and DMA out.

The HW exec time is dominated by the gather's SWDGE descriptor generation
(~1.4us per 128-row chunk); all compute overlaps underneath it.
"""
import os
import sys
import types
import contextlib
import ctypes

import numpy as np

import concourse.bacc as bacc
import concourse.mybir as mybir
import concourse.tile as tile
from concourse.bass import IndirectOffsetOnAxis
from concourse.bass_utils import run_bass_kernel_spmd
from concourse.masks import make_identity

P = 128
N_NODES = 50000
D_IN = 128
D_HID = 256
D_OUT = 128
N_CORES = 8
NB = 49                     # node blocks per core
NPC = NB * P                # nodes per core (6272), 8*6272 = 50176 >= 50000
N_PAD = N_CORES * NPC

F32 = mybir.dt.float32
I32 = mybir.dt.int32

_LAST_EXEC_NS = None        # set when BASS_GNN_TRACE=1


# ---------------------------------------------------------------------------
# NTFF profiling hook (only used when BASS_GNN_TRACE=1); injects the missing
# antenv.axon_hooks module using ctypes against libaxon_pjrt.so.
# ---------------------------------------------------------------------------
def _install_ntff_hook():
    so = "/opt/axon/libaxon_pjrt.so"
    if "antenv.axon_hooks" in sys.modules or not os.path.exists(so):
        return
    lib = ctypes.CDLL(so)
    if not hasattr(lib, "axon_start_nrt_profile"):
        return
    lib.axon_start_nrt_profile.argtypes = [ctypes.POINTER(ctypes.c_int64), ctypes.c_size_t]
    lib.axon_start_nrt_profile.restype = ctypes.c_int64
    lib.axon_stop_nrt_profile.argtypes = [ctypes.c_char_p]
    lib.axon_stop_nrt_profile.restype = ctypes.c_int64

    @contextlib.contextmanager
    def _hook(output_dir, device_ids):
        import jax

        jax.devices()
        if device_ids:
            ids = (ctypes.c_int64 * len(device_ids))(*device_ids)
            rc = lib.axon_start_nrt_profile(ids, len(device_ids))
        else:
            rc = lib.axon_start_nrt_profile(None, 0)
        if rc != 0:
            raise RuntimeError(f"axon_start_nrt_profile rc={rc}")
        try:
            yield
        finally:
            n = lib.axon_stop_nrt_profile(str(output_dir).encode())
            print(f"profile: {n} ntff file(s) -> {output_dir}", file=sys.stderr)

    mod = types.ModuleType("antenv.axon_hooks")
    mod.get_axon_ntff_profile_hook = lambda: _hook
    mod.set_axon_ntff_profile_hook = lambda h: None
    sys.modules["antenv.axon_hooks"] = mod


# ---------------------------------------------------------------------------
# Host-side edge preprocessing
# ---------------------------------------------------------------------------
def _preprocess(edge_index):
    """Bucket doubled edges by destination block; build per-core gather-index
    and local-receiver tile arrays in the [lane p, chunk col] layout.

    Returns (idx_tiles[c], rloc_tiles[c], Kb[49], offs[50]).
    """
    send = np.concatenate([edge_index[0], edge_index[1]]).astype(np.int64)
    recv = np.concatenate([edge_index[1], edge_index[0]]).astype(np.int64)

    blk = recv // P                      # global block id, 0..391
    order = np.argsort(blk, kind="stable")
    send_s = send[order].astype(np.int32)
    recv_s = recv[order]
    blk_s = blk[order]

    n_blk_glob = N_PAD // P              # 392
    counts = np.bincount(blk_s, minlength=n_blk_glob)          # [392]
    counts_cb = counts.reshape(N_CORES, NB)                    # [core, block]
    Kb = np.ceil(counts_cb.max(axis=0) / P).astype(np.int64)   # per-block chunks
    Kb = np.maximum(Kb, 1)
    offs = np.concatenate([[0], np.cumsum(Kb)]).astype(np.int64)
    TOT = int(offs[-1])

    starts = np.concatenate([[0], np.cumsum(counts)])          # per global block
    # rank of each edge within its block
    j = np.arange(send_s.shape[0]) - starts[blk_s]

    idx_tiles, rloc_tiles = [], []
    for c in range(N_CORES):
        lo, hi = starts[c * NB], starts[(c + 1) * NB]
        sl = slice(lo, hi)
        b_local = blk_s[sl] - c * NB
        jj = j[sl]
        col = offs[b_local] + jj // P
        lane = jj % P
        idx_t = np.zeros((P, TOT), dtype=np.int32)
        rloc_t = np.full((P, TOT), -1.0, dtype=np.float32)
        idx_t[lane, col] = send_s[sl]
        rloc_t[lane, col] = (recv_s[sl] - (c * NPC + b_local * P)).astype(np.float32)
        idx_tiles.append(idx_t)
        rloc_tiles.append(rloc_t)
    return idx_tiles, rloc_tiles, Kb, offs


# ---------------------------------------------------------------------------
# Kernel build
# ---------------------------------------------------------------------------
def _build(Kb, offs):
    TOT = int(offs[-1])
    NMETA = P + TOT + 7     # iota | rloc | b1(2) | b2(2) | b3 | ln_g | ln_b
    nc = bacc.Bacc("TRN2", target_bir_lowering=False, debug=False, num_devices=N_CORES)

    x = nc.declare_dram_parameter("x", [N_NODES, D_IN], F32, isOutput=False)
    idx = nc.declare_dram_parameter("idx", [P, TOT], I32, isOutput=False)
    meta = nc.declare_dram_parameter("meta", [P, NMETA], F32, isOutput=False)
    w1 = nc.declare_dram_parameter("w1", [D_IN, D_HID], F32, isOutput=False)
    w2 = nc.declare_dram_parameter("w2", [D_HID, D_HID], F32, isOutput=False)
    w3 = nc.declare_dram_parameter("w3", [D_HID, D_OUT], F32, isOutput=False)
    out = nc.declare_dram_parameter("out", [NPC, D_OUT], F32, isOutput=True)

    AF = mybir.ActivationFunctionType
    OP = mybir.AluOpType

    with tile.TileContext(nc) as tc:
        with (
            tc.tile_pool(name="const", bufs=1) as cpool,
            tc.tile_pool(name="gather", bufs=3) as gpool,
            tc.tile_pool(name="spool", bufs=6) as spool,
            tc.tile_pool(name="agg", bufs=1) as apool,
            tc.tile_pool(name="hid", bufs=10) as hpool,
            tc.tile_pool(name="rows", bufs=8) as rpool,
            tc.tile_pool(name="outp", bufs=4) as opool,
            tc.tile_pool(name="ps1", bufs=2, space="PSUM") as ps1pool,
            tc.tile_pool(name="ps2", bufs=4, space="PSUM") as ps2pool,
            tc.tile_pool(name="psr", bufs=2, space="PSUM") as psrpool,
        ):
            # ---- constants -------------------------------------------------
            idx_sb = cpool.tile([P, TOT], I32)
            nc.sync.dma_start(out=idx_sb[:], in_=idx[:])
            meta_sb = cpool.tile([P, NMETA], F32)
            nc.sync.dma_start(out=meta_sb[:], in_=meta[:])
            iota_sb = meta_sb[:, 0:P]
            rloc_sb = meta_sb[:, P : P + TOT]
            b1_ap = meta_sb[:, P + TOT : P + TOT + 2]
            b2_ap = meta_sb[:, P + TOT + 2 : P + TOT + 4]
            b3_ap = meta_sb[:, P + TOT + 4 : P + TOT + 5]
            lng_ap = meta_sb[:, P + TOT + 5 : P + TOT + 6]
            lnb_ap = meta_sb[:, P + TOT + 6 : P + TOT + 7]

            w1_sb = cpool.tile([P, D_HID], F32)
            nc.sync.dma_start(out=w1_sb[:], in_=w1[:])
            # w2 [256, 256] -> [128, 2, 256]: [:, h*256:(h+1)*256] = w2[h*128:(h+1)*128]
            w2_sb = cpool.tile([P, 2 * D_HID], F32)
            nc.sync.dma_start(
                out=w2_sb[:].rearrange("p (h j) -> p h j", h=2),
                in_=w2[:].rearrange("(h p) j -> p h j", p=P),
            )
            # w3 [256, 128] -> [128, 2, 128]
            w3_sb = cpool.tile([P, 2 * D_OUT], F32)
            nc.sync.dma_start(
                out=w3_sb[:].rearrange("p (h j) -> p h j", h=2),
                in_=w3[:].rearrange("(h p) j -> p h j", p=P),
            )

            ident_sb = cpool.tile([P, P], F32)
            make_identity(nc, ident_sb[:])
            ones_col = cpool.tile([P, 1], F32)
            nc.vector.memset(ones_col[:], 1.0)
            ones_row = cpool.tile([1, P], F32)
            nc.vector.memset(ones_row[:], 1.0)

            aggT = apool.tile([P, NPC], F32)    # [feat, node] for this core

            # ---- phase 1: gather + one-hot segment matmul ------------------
            def phase1_block(b):
                kb = int(Kb[b])
                off = int(offs[b])
                mt = gpool.tile([P, kb * D_IN], F32, tag="m")
                for k in range(kb):
                    nc.gpsimd.indirect_dma_start(
                        out=mt[:, k * D_IN : (k + 1) * D_IN],
                        out_offset=None,
                        in_=x[:],
                        in_offset=IndirectOffsetOnAxis(
                            ap=idx_sb[:, off + k : off + k + 1], axis=0
                        ),
                    )
                s = spool.tile([P, kb * P], F32, tag="s")
                nc.vector.tensor_tensor(
                    out=s[:].rearrange("p (k j) -> p k j", k=kb),
                    in0=iota_sb.unsqueeze(1).broadcast_to([P, kb, P]),
                    in1=rloc_sb[:, off : off + kb].unsqueeze(2).broadcast_to(
                        [P, kb, P]
                    ),
                    op=OP.is_equal,
                )
                ps = ps1pool.tile([P, P], F32, tag="p1")
                for k in range(kb):
                    nc.tensor.matmul(
                        out=ps[:],
                        lhsT=mt[:, k * D_IN : (k + 1) * D_IN],
                        rhs=s[:, k * P : (k + 1) * P],
                        start=(k == 0),
                        stop=(k == kb - 1),
                    )
                nc.scalar.copy(out=aggT[:, b * P : (b + 1) * P], in_=ps[:])

            # ---- phase 2: transposed MLP + LayerNorm -----------------------
            def phase2_group(g0, ng):
                rhs_agg = aggT[:, g0 : g0 + ng]
                h1 = []
                for jh in range(2):
                    p1 = ps2pool.tile([P, ng], F32, tag="p2")
                    nc.tensor.matmul(
                        out=p1[:],
                        lhsT=w1_sb[:, jh * P : (jh + 1) * P],
                        rhs=rhs_agg,
                        start=True,
                        stop=True,
                    )
                    t = hpool.tile([P, ng], F32, tag="h")
                    nc.scalar.activation(t[:], p1[:], AF.Gelu, bias=b1_ap[:, jh : jh + 1])
                    h1.append(t)
                h2 = []
                for kh in range(2):
                    p2 = ps2pool.tile([P, ng], F32, tag="p2")
                    for jh in range(2):
                        nc.tensor.matmul(
                            out=p2[:],
                            lhsT=w2_sb[:, jh * D_HID + kh * P : jh * D_HID + (kh + 1) * P],
                            rhs=h1[jh][:],
                            start=(jh == 0),
                            stop=(jh == 1),
                        )
                    t = hpool.tile([P, ng], F32, tag="h")
                    nc.scalar.activation(t[:], p2[:], AF.Gelu, bias=b2_ap[:, kh : kh + 1])
                    h2.append(t)
                p3 = ps2pool.tile([P, ng], F32, tag="p2")
                for kh in range(2):
                    nc.tensor.matmul(
                        out=p3[:],
                        lhsT=w3_sb[:, kh * D_OUT : (kh + 1) * D_OUT],
                        rhs=h2[kh][:],
                        start=(kh == 0),
                        stop=(kh == 1),
                    )
                h3 = hpool.tile([P, ng], F32, tag="h")
                nc.scalar.activation(h3[:], p3[:], AF.Identity, bias=b3_ap)
                sq = hpool.tile([P, ng], F32, tag="h")
                nc.scalar.activation(sq[:], h3[:], AF.Square)

                mu_ps = psrpool.tile([1, ng], F32, tag="pr")
                nc.tensor.matmul(out=mu_ps[:], lhsT=ones_col[:], rhs=h3[:], start=True, stop=True)
                s2_ps = psrpool.tile([1, ng], F32, tag="pr")
                nc.tensor.matmul(out=s2_ps[:], lhsT=ones_col[:], rhs=sq[:], start=True, stop=True)

                m_row = rpool.tile([1, ng], F32, tag="r")
                nc.vector.tensor_scalar_mul(m_row[:], mu_ps[:], 1.0 / P)
                q_row = rpool.tile([1, ng], F32, tag="r")
                nc.vector.tensor_tensor(out=q_row[:], in0=m_row[:], in1=m_row[:], op=OP.mult)
                v_row = rpool.tile([1, ng], F32, tag="r")
                nc.vector.tensor_scalar_mul(v_row[:], s2_ps[:], 1.0 / P)
                nc.vector.tensor_tensor(out=v_row[:], in0=v_row[:], in1=q_row[:], op=OP.subtract)
                nc.vector.tensor_scalar_add(v_row[:], v_row[:], 1e-5)
                sdev = rpool.tile([1, ng], F32, tag="r")
                nc.scalar.activation(sdev[:], v_row[:], AF.Sqrt)
                inv_row = rpool.tile([1, ng], F32, tag="r")
                with nc.allow_low_precision("matching jax rsqrt f32"):
                    nc.vector.reciprocal(inv_row[:], sdev[:])
                minv_row = rpool.tile([1, ng], F32, tag="r")
                nc.vector.tensor_tensor(out=minv_row[:], in0=m_row[:], in1=inv_row[:], op=OP.mult)

                inv_ps = ps2pool.tile([P, ng], F32, tag="p2")
                nc.tensor.matmul(out=inv_ps[:], lhsT=ones_row[:], rhs=inv_row[:], start=True, stop=True)
                minv_ps = ps2pool.tile([P, ng], F32, tag="p2")
                nc.tensor.matmul(out=minv_ps[:], lhsT=ones_row[:], rhs=minv_row[:], start=True, stop=True)

                t1 = hpool.tile([P, ng], F32, tag="h")
                nc.vector.tensor_tensor(out=t1[:], in0=h3[:], in1=inv_ps[:], op=OP.mult)
                t2 = hpool.tile([P, ng], F32, tag="h")
                nc.vector.tensor_tensor(out=t2[:], in0=t1[:], in1=minv_ps[:], op=OP.subtract)
                oT = hpool.tile([P, ng], F32, tag="h")
                nc.vector.tensor_scalar(
                    out=oT[:], in0=t2[:], scalar1=lng_ap, scalar2=lnb_ap,
                    op0=OP.mult, op1=OP.add,
                )

                for t in range(ng // P):
                    trp = ps2pool.tile([P, P], F32, tag="p2")
                    nc.tensor.transpose(out=trp[:], in_=oT[:, t * P : (t + 1) * P], identity=ident_sb[:])
                    ot = opool.tile([P, P], F32, tag="o")
                    nc.scalar.copy(out=ot[:], in_=trp[:])
                    r0 = g0 + t * P
                    nc.sync.dma_start(out=out[r0 : r0 + P, :], in_=ot[:])

            # interleave phase 2 under the Pool-bound gather stream
            groups = [(g * 512, 512) for g in range(NPC // 512)]
            if NPC % 512:
                groups.append((NPC - NPC % 512, NPC % 512))
            gi = 0
            for b in range(NB):
                phase1_block(b)
                while gi < len(groups) and groups[gi][0] + groups[gi][1] <= (b + 1) * P:
                    phase2_group(*groups[gi])
                    gi += 1
            while gi < len(groups):
                phase2_group(*groups[gi])
                gi += 1
    nc.compile()
    return nc


# ---------------------------------------------------------------------------
# Public entry point
# ---------------------------------------------------------------------------
def kernel(x, edge_index, W1, b1, W2, b2, W3, b3, ln_g, ln_b):
    global _LAST_EXEC_NS
    x = np.ascontiguousarray(np.asarray(x, dtype=np.float32))
    edge_index = np.asarray(edge_index)

    idx_tiles, rloc_tiles, Kb, offs = _preprocess(edge_index)
    TOT = int(offs[-1])

    iota = np.tile(np.arange(P, dtype=np.float32), (P, 1))
    b1_2 = np.asarray(b1, np.float32).reshape(2, P).T          # [128, 2]
    b2_2 = np.asarray(b2, np.float32).reshape(2, P).T
    b3_1 = np.asarray(b3, np.float32).reshape(1, P).T          # [128, 1]
    g_1 = np.asarray(ln_g, np.float32).reshape(1, P).T
    lb_1 = np.asarray(ln_b, np.float32).reshape(1, P).T

    in_maps = []
    for c in range(N_CORES):
        m = np.concatenate(
            [iota, rloc_tiles[c], b1_2, b2_2, b3_1, g_1, lb_1], axis=1
        ).astype(np.float32)
        in_maps.append(
            {
                "x": x,
                "idx": idx_tiles[c],
                "meta": np.ascontiguousarray(m),
                "w1": np.ascontiguousarray(np.asarray(W1, np.float32)),
                "w2": np.ascontiguousarray(np.asarray(W2, np.float32)),
                "w3": np.ascontiguousarray(np.asarray(W3, np.float32)),
            }
        )

    nc = _build(Kb, offs)

    trace = os.environ.get("BASS_GNN_TRACE", "0") == "1"
    if trace:
        _install_ntff_hook()
    r = run_bass_kernel_spmd(nc, in_maps, list(range(N_CORES)), trace=trace)
    _LAST_EXEC_NS = r.exec_time_ns

    full = np.concatenate([r.results[c]["out"] for c in range(N_CORES)], axis=0)
    return np.ascontiguousarray(full[:N_NODES])



# revision 24
# speedup vs baseline: 1.2214x; 1.0545x over previous
"""GNN message passing (nn_NodeToNode) on 8 trn2 NeuronCores via Bass/Tile.

Algorithm (per core, SPMD):
  - Nodes are range-sharded: core c owns nodes [c*6272, (c+1)*6272) (50176 total,
    padded; host slices output back to 50000).
  - Host sorts the doubled edge list by receiver and buckets edges into the
    owner core's 49 node-blocks of 128. Per (core, block) the edge list is
    padded to whole 128-edge chunks (pad: sender=0, rloc=-1).
  - Phase 1 on device: for each chunk, gather 128 sender rows of x (512B each)
    via vector-indirect DMA (one descriptor per row), build the one-hot
    S[e, n] = (iota[n] == rloc[e]) on DVE, and accumulate
    aggT[f, n] += M[e, f]^T . S[e, n] into PSUM over the block's chunks.
    rloc=-1 padding makes S rows zero, masking pad/garbage lanes.
  - Phase 2 on device (transposed layout, per 512-node group): 3-layer MLP with
    per-partition biases on ACT (exact-erf GELU), LayerNorm over the feature
    (=partition) axis via ones-matmul stats + replicate-matmul broadcast,
    then PE transpose back to [node, feat] and DMA out.

The HW exec time is dominated by the gather's SWDGE descriptor generation
(~1.4us per 128-row chunk); all compute overlaps underneath it.
"""
import os
import sys
import types
import contextlib
import ctypes

import numpy as np

import concourse.bacc as bacc
import concourse.mybir as mybir
import concourse.tile as tile
from concourse.bass import IndirectOffsetOnAxis
from concourse.bass_utils import run_bass_kernel_spmd
from concourse.masks import make_identity

P = 128
N_NODES = 50000
D_IN = 128
D_HID = 256
D_OUT = 128
N_CORES = 8
NB = 49                     # node blocks per core
NPC = NB * P                # nodes per core (6272), 8*6272 = 50176 >= 50000
N_PAD = N_CORES * NPC

F32 = mybir.dt.float32
I32 = mybir.dt.int32

_LAST_EXEC_NS = None        # set when BASS_GNN_TRACE=1


# ---------------------------------------------------------------------------
# NTFF profiling hook (only used when BASS_GNN_TRACE=1); injects the missing
# antenv.axon_hooks module using ctypes against libaxon_pjrt.so.
# ---------------------------------------------------------------------------
def _install_ntff_hook():
    so = "/opt/axon/libaxon_pjrt.so"
    if "antenv.axon_hooks" in sys.modules or not os.path.exists(so):
        return
    lib = ctypes.CDLL(so)
    if not hasattr(lib, "axon_start_nrt_profile"):
        return
    lib.axon_start_nrt_profile.argtypes = [ctypes.POINTER(ctypes.c_int64), ctypes.c_size_t]
    lib.axon_start_nrt_profile.restype = ctypes.c_int64
    lib.axon_stop_nrt_profile.argtypes = [ctypes.c_char_p]
    lib.axon_stop_nrt_profile.restype = ctypes.c_int64

    @contextlib.contextmanager
    def _hook(output_dir, device_ids):
        import jax

        jax.devices()
        if device_ids:
            ids = (ctypes.c_int64 * len(device_ids))(*device_ids)
            rc = lib.axon_start_nrt_profile(ids, len(device_ids))
        else:
            rc = lib.axon_start_nrt_profile(None, 0)
        if rc != 0:
            raise RuntimeError(f"axon_start_nrt_profile rc={rc}")
        try:
            yield
        finally:
            n = lib.axon_stop_nrt_profile(str(output_dir).encode())
            print(f"profile: {n} ntff file(s) -> {output_dir}", file=sys.stderr)

    mod = types.ModuleType("antenv.axon_hooks")
    mod.get_axon_ntff_profile_hook = lambda: _hook
    mod.set_axon_ntff_profile_hook = lambda h: None
    sys.modules["antenv.axon_hooks"] = mod


# ---------------------------------------------------------------------------
# Host-side edge preprocessing
# ---------------------------------------------------------------------------
def _preprocess(edge_index):
    """Bucket doubled edges by destination block; build per-core gather-index
    and local-receiver tile arrays in the [lane p, chunk col] layout.

    Returns (idx_tiles[c], rloc_tiles[c], Kb[49], offs[50]).
    """
    send = np.concatenate([edge_index[0], edge_index[1]]).astype(np.int64)
    recv = np.concatenate([edge_index[1], edge_index[0]]).astype(np.int64)

    blk = recv // P                      # global block id, 0..391
    order = np.argsort(blk, kind="stable")
    send_s = send[order].astype(np.int32)
    recv_s = recv[order]
    blk_s = blk[order]

    n_blk_glob = N_PAD // P              # 392
    counts = np.bincount(blk_s, minlength=n_blk_glob)          # [392]
    counts_cb = counts.reshape(N_CORES, NB)                    # [core, block]
    Kb = np.ceil(counts_cb.max(axis=0) / P).astype(np.int64)   # per-block chunks
    Kb = np.maximum(Kb, 1)
    offs = np.concatenate([[0], np.cumsum(Kb)]).astype(np.int64)
    TOT = int(offs[-1])

    starts = np.concatenate([[0], np.cumsum(counts)])          # per global block
    # rank of each edge within its block
    j = np.arange(send_s.shape[0]) - starts[blk_s]

    idx_tiles, rloc_tiles = [], []
    for c in range(N_CORES):
        lo, hi = starts[c * NB], starts[(c + 1) * NB]
        sl = slice(lo, hi)
        b_local = blk_s[sl] - c * NB
        jj = j[sl]
        col = offs[b_local] + jj // P
        lane = jj % P
        idx_t = np.zeros((P, TOT), dtype=np.int32)
        rloc_t = np.full((P, TOT), -1.0, dtype=np.float32)
        idx_t[lane, col] = send_s[sl]
        rloc_t[lane, col] = (recv_s[sl] - (c * NPC + b_local * P)).astype(np.float32)
        idx_tiles.append(idx_t)
        rloc_tiles.append(rloc_t)
    return idx_tiles, rloc_tiles, Kb, offs


# ---------------------------------------------------------------------------
# Kernel build
# ---------------------------------------------------------------------------
def _build(Kb, offs):
    TOT = int(offs[-1])
    NMETA = P + TOT + 7     # iota | rloc | b1(2) | b2(2) | b3 | ln_g | ln_b
    nc = bacc.Bacc("TRN2", target_bir_lowering=False, debug=False, num_devices=N_CORES)

    x = nc.declare_dram_parameter("x", [N_NODES, D_IN], F32, isOutput=False)
    idx = nc.declare_dram_parameter("idx", [P, TOT], I32, isOutput=False)
    meta = nc.declare_dram_parameter("meta", [P, NMETA], F32, isOutput=False)
    w1 = nc.declare_dram_parameter("w1", [D_IN, D_HID], F32, isOutput=False)
    w2 = nc.declare_dram_parameter("w2", [D_HID, D_HID], F32, isOutput=False)
    w3 = nc.declare_dram_parameter("w3", [D_HID, D_OUT], F32, isOutput=False)
    out = nc.declare_dram_parameter("out", [NPC, D_OUT], F32, isOutput=True)

    AF = mybir.ActivationFunctionType
    OP = mybir.AluOpType

    with tile.TileContext(nc) as tc:
        with (
            tc.tile_pool(name="const", bufs=1) as cpool,
            tc.tile_pool(name="gather", bufs=3) as gpool,
            tc.tile_pool(name="spool", bufs=6) as spool,
            tc.tile_pool(name="agg", bufs=1) as apool,
            tc.tile_pool(name="hid", bufs=10) as hpool,
            tc.tile_pool(name="rows", bufs=8) as rpool,
            tc.tile_pool(name="outp", bufs=4) as opool,
            tc.tile_pool(name="ps1", bufs=2, space="PSUM") as ps1pool,
            tc.tile_pool(name="ps2", bufs=4, space="PSUM") as ps2pool,
            tc.tile_pool(name="psr", bufs=2, space="PSUM") as psrpool,
        ):
            # ---- constants -------------------------------------------------
            idx_sb = cpool.tile([P, TOT], I32)
            nc.sync.dma_start(out=idx_sb[:], in_=idx[:])
            meta_sb = cpool.tile([P, NMETA], F32)
            nc.sync.dma_start(out=meta_sb[:], in_=meta[:])
            iota_sb = meta_sb[:, 0:P]
            rloc_sb = meta_sb[:, P : P + TOT]
            b1_ap = meta_sb[:, P + TOT : P + TOT + 2]
            b2_ap = meta_sb[:, P + TOT + 2 : P + TOT + 4]
            b3_ap = meta_sb[:, P + TOT + 4 : P + TOT + 5]
            lng_ap = meta_sb[:, P + TOT + 5 : P + TOT + 6]
            lnb_ap = meta_sb[:, P + TOT + 6 : P + TOT + 7]

            w1_sb = cpool.tile([P, D_HID], F32)
            nc.sync.dma_start(out=w1_sb[:], in_=w1[:])
            # w2 [256, 256] -> [128, 2, 256]: [:, h*256:(h+1)*256] = w2[h*128:(h+1)*128]
            w2_sb = cpool.tile([P, 2 * D_HID], F32)
            nc.sync.dma_start(
                out=w2_sb[:].rearrange("p (h j) -> p h j", h=2),
                in_=w2[:].rearrange("(h p) j -> p h j", p=P),
            )
            # w3 [256, 128] -> [128, 2, 128]
            w3_sb = cpool.tile([P, 2 * D_OUT], F32)
            nc.sync.dma_start(
                out=w3_sb[:].rearrange("p (h j) -> p h j", h=2),
                in_=w3[:].rearrange("(h p) j -> p h j", p=P),
            )

            ident_sb = cpool.tile([P, P], F32)
            make_identity(nc, ident_sb[:])
            ones_col = cpool.tile([P, 1], F32)
            nc.vector.memset(ones_col[:], 1.0)
            ones_row = cpool.tile([1, P], F32)
            nc.vector.memset(ones_row[:], 1.0)

            aggT = apool.tile([P, NPC], F32)    # [feat, node] for this core

            # ---- phase 1: gather + one-hot segment matmul ------------------
            for b in range(NB):
                kb = int(Kb[b])
                off = int(offs[b])
                mt = gpool.tile([P, kb * D_IN], F32, tag="m")
                for k in range(kb):
                    nc.gpsimd.indirect_dma_start(
                        out=mt[:, k * D_IN : (k + 1) * D_IN],
                        out_offset=None,
                        in_=x[:],
                        in_offset=IndirectOffsetOnAxis(
                            ap=idx_sb[:, off + k : off + k + 1], axis=0
                        ),
                    )
                ps = ps1pool.tile([P, P], F32, tag="p1")
                for k in range(kb):
                    s = spool.tile([P, P], F32, tag="s")
                    nc.vector.tensor_scalar(
                        out=s[:],
                        in0=iota_sb,
                        scalar1=rloc_sb[:, off + k : off + k + 1],
                        scalar2=None,
                        op0=OP.is_equal,
                    )
                    nc.tensor.matmul(
                        out=ps[:],
                        lhsT=mt[:, k * D_IN : (k + 1) * D_IN],
                        rhs=s[:],
                        start=(k == 0),
                        stop=(k == kb - 1),
                    )
                nc.scalar.copy(out=aggT[:, b * P : (b + 1) * P], in_=ps[:])

            # ---- phase 2: transposed MLP + LayerNorm -----------------------
            groups = [(g * 512, 512) for g in range(NPC // 512)]
            if NPC % 512:
                groups.append((NPC - NPC % 512, NPC % 512))
            for g0, ng in groups:
                rhs_agg = aggT[:, g0 : g0 + ng]
                h1 = []
                for jh in range(2):
                    p1 = ps2pool.tile([P, ng], F32, tag="p2")
                    nc.tensor.matmul(
                        out=p1[:],
                        lhsT=w1_sb[:, jh * P : (jh + 1) * P],
                        rhs=rhs_agg,
                        start=True,
                        stop=True,
                    )
                    t = hpool.tile([P, ng], F32, tag="h")
                    nc.scalar.activation(t[:], p1[:], AF.Gelu, bias=b1_ap[:, jh : jh + 1])
                    h1.append(t)
                h2 = []
                for kh in range(2):
                    p2 = ps2pool.tile([P, ng], F32, tag="p2")
                    for jh in range(2):
                        nc.tensor.matmul(
                            out=p2[:],
                            lhsT=w2_sb[:, jh * D_HID + kh * P : jh * D_HID + (kh + 1) * P],
                            rhs=h1[jh][:],
                            start=(jh == 0),
                            stop=(jh == 1),
                        )
                    t = hpool.tile([P, ng], F32, tag="h")
                    nc.scalar.activation(t[:], p2[:], AF.Gelu, bias=b2_ap[:, kh : kh + 1])
                    h2.append(t)
                p3 = ps2pool.tile([P, ng], F32, tag="p2")
                for kh in range(2):
                    nc.tensor.matmul(
                        out=p3[:],
                        lhsT=w3_sb[:, kh * D_OUT : (kh + 1) * D_OUT],
                        rhs=h2[kh][:],
                        start=(kh == 0),
                        stop=(kh == 1),
                    )
                h3 = hpool.tile([P, ng], F32, tag="h")
                nc.scalar.activation(h3[:], p3[:], AF.Identity, bias=b3_ap)
                sq = hpool.tile([P, ng], F32, tag="h")
                nc.scalar.activation(sq[:], h3[:], AF.Square)

                mu_ps = psrpool.tile([1, ng], F32, tag="pr")
                nc.tensor.matmul(out=mu_ps[:], lhsT=ones_col[:], rhs=h3[:], start=True, stop=True)
                s2_ps = psrpool.tile([1, ng], F32, tag="pr")
                nc.tensor.matmul(out=s2_ps[:], lhsT=ones_col[:], rhs=sq[:], start=True, stop=True)

                m_row = rpool.tile([1, ng], F32, tag="r")
                nc.vector.tensor_scalar_mul(m_row[:], mu_ps[:], 1.0 / P)
                q_row = rpool.tile([1, ng], F32, tag="r")
                nc.vector.tensor_tensor(out=q_row[:], in0=m_row[:], in1=m_row[:], op=OP.mult)
                v_row = rpool.tile([1, ng], F32, tag="r")
                nc.vector.tensor_scalar_mul(v_row[:], s2_ps[:], 1.0 / P)
                nc.vector.tensor_tensor(out=v_row[:], in0=v_row[:], in1=q_row[:], op=OP.subtract)
                nc.vector.tensor_scalar_add(v_row[:], v_row[:], 1e-5)
                sdev = rpool.tile([1, ng], F32, tag="r")
                nc.scalar.activation(sdev[:], v_row[:], AF.Sqrt)
                inv_row = rpool.tile([1, ng], F32, tag="r")
                with nc.allow_low_precision("matching jax rsqrt f32"):
                    nc.vector.reciprocal(inv_row[:], sdev[:])
                minv_row = rpool.tile([1, ng], F32, tag="r")
                nc.vector.tensor_tensor(out=minv_row[:], in0=m_row[:], in1=inv_row[:], op=OP.mult)

                inv_ps = ps2pool.tile([P, ng], F32, tag="p2")
                nc.tensor.matmul(out=inv_ps[:], lhsT=ones_row[:], rhs=inv_row[:], start=True, stop=True)
                minv_ps = ps2pool.tile([P, ng], F32, tag="p2")
                nc.tensor.matmul(out=minv_ps[:], lhsT=ones_row[:], rhs=minv_row[:], start=True, stop=True)

                t1 = hpool.tile([P, ng], F32, tag="h")
                nc.vector.tensor_tensor(out=t1[:], in0=h3[:], in1=inv_ps[:], op=OP.mult)
                t2 = hpool.tile([P, ng], F32, tag="h")
                nc.vector.tensor_tensor(out=t2[:], in0=t1[:], in1=minv_ps[:], op=OP.subtract)
                oT = hpool.tile([P, ng], F32, tag="h")
                nc.vector.tensor_scalar(
                    out=oT[:], in0=t2[:], scalar1=lng_ap, scalar2=lnb_ap,
                    op0=OP.mult, op1=OP.add,
                )

                for t in range(ng // P):
                    trp = ps2pool.tile([P, P], F32, tag="p2")
                    nc.tensor.transpose(out=trp[:], in_=oT[:, t * P : (t + 1) * P], identity=ident_sb[:])
                    ot = opool.tile([P, P], F32, tag="o")
                    nc.scalar.copy(out=ot[:], in_=trp[:])
                    r0 = g0 + t * P
                    nc.sync.dma_start(out=out[r0 : r0 + P, :], in_=ot[:])
    nc.compile()
    return nc


# ---------------------------------------------------------------------------
# Public entry point
# ---------------------------------------------------------------------------
def kernel(x, edge_index, W1, b1, W2, b2, W3, b3, ln_g, ln_b):
    global _LAST_EXEC_NS
    x = np.ascontiguousarray(np.asarray(x, dtype=np.float32))
    edge_index = np.asarray(edge_index)

    idx_tiles, rloc_tiles, Kb, offs = _preprocess(edge_index)
    TOT = int(offs[-1])

    iota = np.tile(np.arange(P, dtype=np.float32), (P, 1))
    b1_2 = np.asarray(b1, np.float32).reshape(2, P).T          # [128, 2]
    b2_2 = np.asarray(b2, np.float32).reshape(2, P).T
    b3_1 = np.asarray(b3, np.float32).reshape(1, P).T          # [128, 1]
    g_1 = np.asarray(ln_g, np.float32).reshape(1, P).T
    lb_1 = np.asarray(ln_b, np.float32).reshape(1, P).T

    in_maps = []
    for c in range(N_CORES):
        m = np.concatenate(
            [iota, rloc_tiles[c], b1_2, b2_2, b3_1, g_1, lb_1], axis=1
        ).astype(np.float32)
        in_maps.append(
            {
                "x": x,
                "idx": idx_tiles[c],
                "meta": np.ascontiguousarray(m),
                "w1": np.ascontiguousarray(np.asarray(W1, np.float32)),
                "w2": np.ascontiguousarray(np.asarray(W2, np.float32)),
                "w3": np.ascontiguousarray(np.asarray(W3, np.float32)),
            }
        )

    nc = _build(Kb, offs)

    trace = os.environ.get("BASS_GNN_TRACE", "0") == "1"
    if trace:
        _install_ntff_hook()
    r = run_bass_kernel_spmd(nc, in_maps, list(range(N_CORES)), trace=trace)
    _LAST_EXEC_NS = r.exec_time_ns

    full = np.concatenate([r.results[c]["out"] for c in range(N_CORES)], axis=0)
    return np.ascontiguousarray(full[:N_NODES])



# revision 25
# speedup vs baseline: 1.2239x; 1.0021x over previous
"""GNN message passing (nn_NodeToNode) on 8 trn2 NeuronCores via Bass/Tile.

Algorithm (per core, SPMD):
  - Nodes are range-sharded: core c owns nodes [c*6272, (c+1)*6272) (50176 total,
    padded; host slices output back to 50000).
  - Host sorts the doubled edge list by receiver and buckets edges into the
    owner core's 49 node-blocks of 128. Per (core, block) the edge list is
    padded to whole 128-edge chunks (pad: sender=0, rloc=-1).
  - Phase 1 on device: for each chunk, gather 128 sender rows of x (512B each)
    via vector-indirect DMA (one descriptor per row), build the one-hot
    S[e, n] = (iota[n] == rloc[e]) on DVE, and accumulate
    aggT[f, n] += M[e, f]^T . S[e, n] into PSUM over the block's chunks.
    rloc=-1 padding makes S rows zero, masking pad/garbage lanes.
  - Phase 2 on device (transposed layout, per 512-node group): 3-layer MLP with
    per-partition biases on ACT (exact-erf GELU), LayerNorm over the feature
    (=partition) axis via ones-matmul stats + replicate-matmul broadcast,
    then PE transpose back to [node, feat] and DMA out.

The HW exec time is dominated by the gather's SWDGE descriptor generation
(~1.4us per 128-row chunk); all compute overlaps underneath it.
"""
import os
import sys
import types
import contextlib
import ctypes

import numpy as np

import concourse.bacc as bacc
import concourse.mybir as mybir
import concourse.tile as tile
from concourse.bass import IndirectOffsetOnAxis
from concourse.bass_utils import run_bass_kernel_spmd
from concourse.masks import make_identity

P = 128
N_NODES = 50000
D_IN = 128
D_HID = 256
D_OUT = 128
N_CORES = 8
NB = 49                     # node blocks per core
NPC = NB * P                # nodes per core (6272), 8*6272 = 50176 >= 50000
N_PAD = N_CORES * NPC

F32 = mybir.dt.float32
I32 = mybir.dt.int32

_LAST_EXEC_NS = None        # set when BASS_GNN_TRACE=1


# ---------------------------------------------------------------------------
# NTFF profiling hook (only used when BASS_GNN_TRACE=1); injects the missing
# antenv.axon_hooks module using ctypes against libaxon_pjrt.so.
# ---------------------------------------------------------------------------
def _install_ntff_hook():
    so = "/opt/axon/libaxon_pjrt.so"
    if "antenv.axon_hooks" in sys.modules or not os.path.exists(so):
        return
    lib = ctypes.CDLL(so)
    if not hasattr(lib, "axon_start_nrt_profile"):
        return
    lib.axon_start_nrt_profile.argtypes = [ctypes.POINTER(ctypes.c_int64), ctypes.c_size_t]
    lib.axon_start_nrt_profile.restype = ctypes.c_int64
    lib.axon_stop_nrt_profile.argtypes = [ctypes.c_char_p]
    lib.axon_stop_nrt_profile.restype = ctypes.c_int64

    @contextlib.contextmanager
    def _hook(output_dir, device_ids):
        import jax

        jax.devices()
        if device_ids:
            ids = (ctypes.c_int64 * len(device_ids))(*device_ids)
            rc = lib.axon_start_nrt_profile(ids, len(device_ids))
        else:
            rc = lib.axon_start_nrt_profile(None, 0)
        if rc != 0:
            raise RuntimeError(f"axon_start_nrt_profile rc={rc}")
        try:
            yield
        finally:
            n = lib.axon_stop_nrt_profile(str(output_dir).encode())
            print(f"profile: {n} ntff file(s) -> {output_dir}", file=sys.stderr)

    mod = types.ModuleType("antenv.axon_hooks")
    mod.get_axon_ntff_profile_hook = lambda: _hook
    mod.set_axon_ntff_profile_hook = lambda h: None
    sys.modules["antenv.axon_hooks"] = mod


# ---------------------------------------------------------------------------
# Host-side edge preprocessing
# ---------------------------------------------------------------------------
def _preprocess(edge_index):
    """Bucket doubled edges by destination block; build per-core gather-index
    and local-receiver tile arrays in the [lane p, chunk col] layout.

    Returns (idx_tiles[c], rloc_tiles[c], Kb[49], offs[50]).
    """
    send = np.concatenate([edge_index[0], edge_index[1]]).astype(np.int64)
    recv = np.concatenate([edge_index[1], edge_index[0]]).astype(np.int64)

    blk = recv // P                      # global block id, 0..391
    order = np.argsort(blk, kind="stable")
    send_s = send[order].astype(np.int32)
    recv_s = recv[order]
    blk_s = blk[order]

    n_blk_glob = N_PAD // P              # 392
    counts = np.bincount(blk_s, minlength=n_blk_glob)          # [392]
    counts_cb = counts.reshape(N_CORES, NB)                    # [core, block]
    Kb = np.ceil(counts_cb.max(axis=0) / P).astype(np.int64)   # per-block chunks
    Kb = np.maximum(Kb, 1)
    offs = np.concatenate([[0], np.cumsum(Kb)]).astype(np.int64)
    TOT = int(offs[-1])

    starts = np.concatenate([[0], np.cumsum(counts)])          # per global block
    # rank of each edge within its block
    j = np.arange(send_s.shape[0]) - starts[blk_s]

    idx_tiles, rloc_tiles = [], []
    for c in range(N_CORES):
        lo, hi = starts[c * NB], starts[(c + 1) * NB]
        sl = slice(lo, hi)
        b_local = blk_s[sl] - c * NB
        jj = j[sl]
        col = offs[b_local] + jj // P
        lane = jj % P
        idx_t = np.zeros((P, TOT), dtype=np.int32)
        rloc_t = np.full((P, TOT), -1.0, dtype=np.float32)
        idx_t[lane, col] = send_s[sl]
        rloc_t[lane, col] = (recv_s[sl] - (c * NPC + b_local * P)).astype(np.float32)
        idx_tiles.append(idx_t)
        rloc_tiles.append(rloc_t)
    return idx_tiles, rloc_tiles, Kb, offs


# ---------------------------------------------------------------------------
# Kernel build
# ---------------------------------------------------------------------------
def _build(Kb, offs):
    TOT = int(offs[-1])
    NMETA = P + TOT + 7     # iota | rloc | b1(2) | b2(2) | b3 | ln_g | ln_b
    nc = bacc.Bacc("TRN2", target_bir_lowering=False, debug=False, num_devices=N_CORES)

    x = nc.declare_dram_parameter("x", [N_NODES, D_IN], F32, isOutput=False)
    idx = nc.declare_dram_parameter("idx", [P, TOT], I32, isOutput=False)
    meta = nc.declare_dram_parameter("meta", [P, NMETA], F32, isOutput=False)
    w1 = nc.declare_dram_parameter("w1", [D_IN, D_HID], F32, isOutput=False)
    w2 = nc.declare_dram_parameter("w2", [D_HID, D_HID], F32, isOutput=False)
    w3 = nc.declare_dram_parameter("w3", [D_HID, D_OUT], F32, isOutput=False)
    out = nc.declare_dram_parameter("out", [NPC, D_OUT], F32, isOutput=True)

    AF = mybir.ActivationFunctionType
    OP = mybir.AluOpType

    with tile.TileContext(nc) as tc:
        with (
            tc.tile_pool(name="const", bufs=1) as cpool,
            tc.tile_pool(name="gather", bufs=3) as gpool,
            tc.tile_pool(name="spool", bufs=6) as spool,
            tc.tile_pool(name="agg", bufs=1) as apool,
            tc.tile_pool(name="hid", bufs=10) as hpool,
            tc.tile_pool(name="rows", bufs=8) as rpool,
            tc.tile_pool(name="outp", bufs=4) as opool,
            tc.tile_pool(name="ps1", bufs=2, space="PSUM") as ps1pool,
            tc.tile_pool(name="ps2", bufs=4, space="PSUM") as ps2pool,
            tc.tile_pool(name="psr", bufs=2, space="PSUM") as psrpool,
        ):
            # ---- constants -------------------------------------------------
            idx_sb = cpool.tile([P, TOT], I32)
            nc.sync.dma_start(out=idx_sb[:], in_=idx[:])
            meta_sb = cpool.tile([P, NMETA], F32)
            nc.sync.dma_start(out=meta_sb[:], in_=meta[:])
            iota_sb = meta_sb[:, 0:P]
            rloc_sb = meta_sb[:, P : P + TOT]
            b1_ap = meta_sb[:, P + TOT : P + TOT + 2]
            b2_ap = meta_sb[:, P + TOT + 2 : P + TOT + 4]
            b3_ap = meta_sb[:, P + TOT + 4 : P + TOT + 5]
            lng_ap = meta_sb[:, P + TOT + 5 : P + TOT + 6]
            lnb_ap = meta_sb[:, P + TOT + 6 : P + TOT + 7]

            w1_sb = cpool.tile([P, D_HID], F32)
            nc.sync.dma_start(out=w1_sb[:], in_=w1[:])
            # w2 [256, 256] -> [128, 2, 256]: [:, h*256:(h+1)*256] = w2[h*128:(h+1)*128]
            w2_sb = cpool.tile([P, 2 * D_HID], F32)
            nc.sync.dma_start(
                out=w2_sb[:].rearrange("p (h j) -> p h j", h=2),
                in_=w2[:].rearrange("(h p) j -> p h j", p=P),
            )
            # w3 [256, 128] -> [128, 2, 128]
            w3_sb = cpool.tile([P, 2 * D_OUT], F32)
            nc.sync.dma_start(
                out=w3_sb[:].rearrange("p (h j) -> p h j", h=2),
                in_=w3[:].rearrange("(h p) j -> p h j", p=P),
            )

            ident_sb = cpool.tile([P, P], F32)
            make_identity(nc, ident_sb[:])
            ones_col = cpool.tile([P, 1], F32)
            nc.vector.memset(ones_col[:], 1.0)
            ones_row = cpool.tile([1, P], F32)
            nc.vector.memset(ones_row[:], 1.0)

            aggT = apool.tile([P, NPC], F32)    # [feat, node] for this core

            # ---- phase 1: gather + one-hot segment matmul ------------------
            def phase1_block(b):
                kb = int(Kb[b])
                off = int(offs[b])
                mt = gpool.tile([P, kb * D_IN], F32, tag="m")
                for k in range(kb):
                    nc.gpsimd.indirect_dma_start(
                        out=mt[:, k * D_IN : (k + 1) * D_IN],
                        out_offset=None,
                        in_=x[:],
                        in_offset=IndirectOffsetOnAxis(
                            ap=idx_sb[:, off + k : off + k + 1], axis=0
                        ),
                    )
                ps = ps1pool.tile([P, P], F32, tag="p1")
                for k in range(kb):
                    s = spool.tile([P, P], F32, tag="s")
                    nc.vector.tensor_scalar(
                        out=s[:],
                        in0=iota_sb,
                        scalar1=rloc_sb[:, off + k : off + k + 1],
                        scalar2=None,
                        op0=OP.is_equal,
                    )
                    nc.tensor.matmul(
                        out=ps[:],
                        lhsT=mt[:, k * D_IN : (k + 1) * D_IN],
                        rhs=s[:],
                        start=(k == 0),
                        stop=(k == kb - 1),
                    )
                nc.scalar.copy(out=aggT[:, b * P : (b + 1) * P], in_=ps[:])

            # ---- phase 2: transposed MLP + LayerNorm -----------------------
            def phase2_group(g0, ng):
                rhs_agg = aggT[:, g0 : g0 + ng]
                h1 = []
                for jh in range(2):
                    p1 = ps2pool.tile([P, ng], F32, tag="p2")
                    nc.tensor.matmul(
                        out=p1[:],
                        lhsT=w1_sb[:, jh * P : (jh + 1) * P],
                        rhs=rhs_agg,
                        start=True,
                        stop=True,
                    )
                    t = hpool.tile([P, ng], F32, tag="h")
                    nc.scalar.activation(t[:], p1[:], AF.Gelu, bias=b1_ap[:, jh : jh + 1])
                    h1.append(t)
                h2 = []
                for kh in range(2):
                    p2 = ps2pool.tile([P, ng], F32, tag="p2")
                    for jh in range(2):
                        nc.tensor.matmul(
                            out=p2[:],
                            lhsT=w2_sb[:, jh * D_HID + kh * P : jh * D_HID + (kh + 1) * P],
                            rhs=h1[jh][:],
                            start=(jh == 0),
                            stop=(jh == 1),
                        )
                    t = hpool.tile([P, ng], F32, tag="h")
                    nc.scalar.activation(t[:], p2[:], AF.Gelu, bias=b2_ap[:, kh : kh + 1])
                    h2.append(t)
                p3 = ps2pool.tile([P, ng], F32, tag="p2")
                for kh in range(2):
                    nc.tensor.matmul(
                        out=p3[:],
                        lhsT=w3_sb[:, kh * D_OUT : (kh + 1) * D_OUT],
                        rhs=h2[kh][:],
                        start=(kh == 0),
                        stop=(kh == 1),
                    )
                h3 = hpool.tile([P, ng], F32, tag="h")
                nc.scalar.activation(h3[:], p3[:], AF.Identity, bias=b3_ap)
                sq = hpool.tile([P, ng], F32, tag="h")
                nc.scalar.activation(sq[:], h3[:], AF.Square)

                mu_ps = psrpool.tile([1, ng], F32, tag="pr")
                nc.tensor.matmul(out=mu_ps[:], lhsT=ones_col[:], rhs=h3[:], start=True, stop=True)
                s2_ps = psrpool.tile([1, ng], F32, tag="pr")
                nc.tensor.matmul(out=s2_ps[:], lhsT=ones_col[:], rhs=sq[:], start=True, stop=True)

                m_row = rpool.tile([1, ng], F32, tag="r")
                nc.vector.tensor_scalar_mul(m_row[:], mu_ps[:], 1.0 / P)
                q_row = rpool.tile([1, ng], F32, tag="r")
                nc.vector.tensor_tensor(out=q_row[:], in0=m_row[:], in1=m_row[:], op=OP.mult)
                v_row = rpool.tile([1, ng], F32, tag="r")
                nc.vector.tensor_scalar_mul(v_row[:], s2_ps[:], 1.0 / P)
                nc.vector.tensor_tensor(out=v_row[:], in0=v_row[:], in1=q_row[:], op=OP.subtract)
                nc.vector.tensor_scalar_add(v_row[:], v_row[:], 1e-5)
                sdev = rpool.tile([1, ng], F32, tag="r")
                nc.scalar.activation(sdev[:], v_row[:], AF.Sqrt)
                inv_row = rpool.tile([1, ng], F32, tag="r")
                with nc.allow_low_precision("matching jax rsqrt f32"):
                    nc.vector.reciprocal(inv_row[:], sdev[:])
                minv_row = rpool.tile([1, ng], F32, tag="r")
                nc.vector.tensor_tensor(out=minv_row[:], in0=m_row[:], in1=inv_row[:], op=OP.mult)

                inv_ps = ps2pool.tile([P, ng], F32, tag="p2")
                nc.tensor.matmul(out=inv_ps[:], lhsT=ones_row[:], rhs=inv_row[:], start=True, stop=True)
                minv_ps = ps2pool.tile([P, ng], F32, tag="p2")
                nc.tensor.matmul(out=minv_ps[:], lhsT=ones_row[:], rhs=minv_row[:], start=True, stop=True)

                t1 = hpool.tile([P, ng], F32, tag="h")
                nc.vector.tensor_tensor(out=t1[:], in0=h3[:], in1=inv_ps[:], op=OP.mult)
                t2 = hpool.tile([P, ng], F32, tag="h")
                nc.vector.tensor_tensor(out=t2[:], in0=t1[:], in1=minv_ps[:], op=OP.subtract)
                oT = hpool.tile([P, ng], F32, tag="h")
                nc.vector.tensor_scalar(
                    out=oT[:], in0=t2[:], scalar1=lng_ap, scalar2=lnb_ap,
                    op0=OP.mult, op1=OP.add,
                )

                for t in range(ng // P):
                    trp = ps2pool.tile([P, P], F32, tag="p2")
                    nc.tensor.transpose(out=trp[:], in_=oT[:, t * P : (t + 1) * P], identity=ident_sb[:])
                    ot = opool.tile([P, P], F32, tag="o")
                    nc.scalar.copy(out=ot[:], in_=trp[:])
                    r0 = g0 + t * P
                    nc.sync.dma_start(out=out[r0 : r0 + P, :], in_=ot[:])

            # emit each 512-node MLP group right after its 4 gather blocks so
            # the PE/ACT/DVE work overlaps under the Pool-bound gather stream
            groups = [(g * 512, 512) for g in range(NPC // 512)]
            if NPC % 512:
                groups.append((NPC - NPC % 512, NPC % 512))
            gi = 0
            for b in range(NB):
                phase1_block(b)
                while gi < len(groups) and groups[gi][0] + groups[gi][1] <= (b + 1) * P:
                    phase2_group(*groups[gi])
                    gi += 1
            while gi < len(groups):
                phase2_group(*groups[gi])
                gi += 1
    nc.compile()
    return nc


# ---------------------------------------------------------------------------
# Public entry point
# ---------------------------------------------------------------------------
def kernel(x, edge_index, W1, b1, W2, b2, W3, b3, ln_g, ln_b):
    global _LAST_EXEC_NS
    x = np.ascontiguousarray(np.asarray(x, dtype=np.float32))
    edge_index = np.asarray(edge_index)

    idx_tiles, rloc_tiles, Kb, offs = _preprocess(edge_index)
    TOT = int(offs[-1])

    iota = np.tile(np.arange(P, dtype=np.float32), (P, 1))
    b1_2 = np.asarray(b1, np.float32).reshape(2, P).T          # [128, 2]
    b2_2 = np.asarray(b2, np.float32).reshape(2, P).T
    b3_1 = np.asarray(b3, np.float32).reshape(1, P).T          # [128, 1]
    g_1 = np.asarray(ln_g, np.float32).reshape(1, P).T
    lb_1 = np.asarray(ln_b, np.float32).reshape(1, P).T

    in_maps = []
    for c in range(N_CORES):
        m = np.concatenate(
            [iota, rloc_tiles[c], b1_2, b2_2, b3_1, g_1, lb_1], axis=1
        ).astype(np.float32)
        in_maps.append(
            {
                "x": x,
                "idx": idx_tiles[c],
                "meta": np.ascontiguousarray(m),
                "w1": np.ascontiguousarray(np.asarray(W1, np.float32)),
                "w2": np.ascontiguousarray(np.asarray(W2, np.float32)),
                "w3": np.ascontiguousarray(np.asarray(W3, np.float32)),
            }
        )

    nc = _build(Kb, offs)

    trace = os.environ.get("BASS_GNN_TRACE", "0") == "1"
    if trace:
        _install_ntff_hook()
    r = run_bass_kernel_spmd(nc, in_maps, list(range(N_CORES)), trace=trace)
    _LAST_EXEC_NS = r.exec_time_ns

    full = np.concatenate([r.results[c]["out"] for c in range(N_CORES)], axis=0)
    return np.ascontiguousarray(full[:N_NODES])

